# revision 6
# baseline (speedup 1.0000x reference)
"""AttentionBlock (GroupNorm + 8-head attention + proj + residual) on 8 TRN2 NeuronCores.

ACT-bound pipeline design. Data-parallel over batch (2 per core, no
collectives). The per-core floor is the softmax exp stream on the scalar
(ACT) engine: 16 (batch,head) x T^2 = 16.8M exps = 128 x [128,1024]
activations ~ 133us; everything else is scheduled to hide under it.

  - ACT runs ONLY Exp (+ a few Identity/Copy ops during the idle startup
    window; all share one act table -> a single table load).  GroupNorm
    rstd avoids ACT Sqrt via a bit-hack rsqrt + Newton step on Pool/DVE.
  - All steady-state PSUM evictions run on DVE (tensor_scalar with
    per-partition AP scalars); Pool takes the GN small-op chains, the
    xh(b1) eviction, softmax-reciprocal broadcasts, and tail residual adds.
  - fp8e4 DoubleRow matmuls (0.5 cyc/row, 2x contraction per instr) for
    qkv, AV and proj; QK^T stays bf16.  DoubleRow weight slices need
    16B-aligned strides -> V^T pads each head to VT_W=66 columns.
  - exp outputs fp8 with a -2.5 logit bias (e4m3 range); softmax ratios are
    bias-invariant.  A ones-column in V^T makes the AV matmul accumulate
    softmax denominators for free; normalization uses partition-shifted DVE
    reciprocal/multiply (verified on HW) -> no SBUF-shuffle DMAs.
  - V bias and proj bias fold host-side: softmax weights sum to 1, so
    out = proj(sum w v) + (x + bp + Wp bv); the residual is pre-biased.
  - PSUM: 2x[128,1024] banks are reserved for the attention S-tiles; all
    interleaved work (qkv(b1), proj(b0), GN) evicts from a separate
    single-bank pool so it never gates the S rotation; AV accumulators
    rotate through 3 single-bank tiles.
  - Emission is one flat software-pipelined stream: 64 QK pairs with AV
    lagging 3 pairs, interleaved units (b1 GN/qkv, proj(b0), residual
    loads) placed by deadline, per-half proj tail on ACT+Pool/DVE.
"""

import numpy as np
from contextlib import ExitStack

import concourse.bass as bass
import concourse.tile as tile
from concourse import mybir
from concourse.bass_utils import run_bass_kernel_spmd

B, C, T = 16, 512, 1024
NH, CH = 8, 64
GS = 16  # channels per GroupNorm group
EPS = 1e-5
NCORES = 8
BL = B // NCORES  # batches per core
P = 128
F32 = mybir.dt.float32
BF16 = mybir.dt.bfloat16
FP8 = mybir.dt.float8e4
AF = mybir.ActivationFunctionType
OP = mybir.AluOpType
DR = mybir.MatmulPerfMode.DoubleRow

VT_W = 66  # per-head V^T columns: 64 ch + 1 ones col + 1 pad so the
# DoubleRow s-pair stride (NH*VT_W fp8 bytes) is 16B-aligned (HW requirement)
VT_USED = 65  # columns actually consumed by the AV matmul
WSCALE = 16.0  # fp8 weight scale (folded back out at PSUM eviction)
EXP_BIAS = -2.5  # logit shift for fp8 exp range; softmax-invariant


U32 = mybir.dt.uint32
RSQRT_MAGIC = 0x5F3759DF


def _gn_phase1(nc, tc, pools, xf, b, js, consts):
    """DVE-only GroupNorm stats for batch b, c-tiles `js`: returns the
    per-channel (mean | E[x^2]) tile m2."""
    gn_pool = pools["gn"]
    nj = len(js)
    bnraw = gn_pool.tile([P, nj, 2, 6], F32, tag="bnraw")
    mv = gn_pool.tile([P, nj, 2], F32, tag="mv")
    for ji, j in enumerate(js):
        for hf in range(2):
            nc.vector.bn_stats(
                out=bnraw[:, ji, hf, :], in_=xf[:, b, j, 512 * hf : 512 * (hf + 1)]
            )
        nc.vector.bn_aggr(out=mv[:, ji, :], in_=bnraw[:, ji, :, :])
    # m2: cols 0:nj per-channel mean (per c-tile), nj:2nj per-channel E[x^2]
    m2 = gn_pool.tile([P, 2 * nj], F32, tag="m2")
    nc.vector.tensor_copy(out=m2[:, 0:nj], in_=mv[:, :, 0])
    nc.vector.tensor_mul(out=m2[:, nj:], in0=mv[:, :, 0], in1=mv[:, :, 0])
    nc.vector.tensor_add(out=m2[:, nj:], in0=m2[:, nj:], in1=mv[:, :, 1])
    return m2


def _gn_phase2(
    nc, tc, pools, b, js, m2, at, bt, consts, chain_eng="pool", copy_eng="vector"
):
    """Group aggregation (PE) + rstd via bit-hack rsqrt + 1 Newton step.
    The serial small-op chain runs on Pool or DVE (chain_eng) so the two
    b0 half-chains execute in parallel.  ACT stays exp-only."""
    gn_pool, psA = pools["gn"], pools["psA"]
    gscale_sb, gbias_sb, gsel_sb, gexp_sb, eps_sb, magic_sb, nrA_sb, nrB_sb = consts
    nj = len(js)
    po = nc.gpsimd if chain_eng == "pool" else nc.vector

    # group-aggregate across the 16-channel groups (partition dim) on PE;
    # gsel carries the 1/16 group mean scaling
    psmm = pools.get("psu") or psA
    shape = [P, 512] if "psu" in pools else [P, T]
    tag = "u" if "psu" in pools else "mm"
    gst_ps = psmm.tile(shape, F32, tag=tag, name=f"gnst_{b}")
    nc.tensor.matmul(
        out=gst_ps[0:8, 0 : 2 * nj], lhsT=gsel_sb, rhs=m2, start=True, stop=True
    )
    gs = gn_pool.tile([8, 2 * nj], F32, tag="gs")  # cols 0:nj mu_g, nj: E2_g
    if copy_eng == "scalar":
        nc.scalar.activation(out=gs, in_=gst_ps[0:8, 0 : 2 * nj], func=AF.Copy)
    else:
        nc.vector.tensor_copy(out=gs, in_=gst_ps[0:8, 0 : 2 * nj])
    musq = gn_pool.tile([8, nj], F32, tag="musq")
    po.tensor_mul(out=musq, in0=gs[:, 0:nj], in1=gs[:, 0:nj])
    vpe = gn_pool.tile([8, nj], F32, tag="vpe")
    po.tensor_sub(out=vpe, in0=gs[:, nj:], in1=musq)
    po.tensor_scalar_add(out=vpe, in0=vpe, scalar1=eps_sb)
    # rstd = rsqrt(vpe): exponent bit-hack seed + 1 Newton-Raphson step.
    # Immediate-scalar tensor_scalar is not supported on Pool -> DVE.
    sh = gn_pool.tile([8, nj], U32, tag="sh")
    nc.vector.tensor_scalar(
        out=sh, in0=vpe.bitcast(U32), scalar1=1, scalar2=None, op0=OP.arith_shift_right
    )
    y0 = gn_pool.tile([8, nj], F32, tag="y0")
    po.tensor_tensor(
        out=y0.bitcast(U32), in0=magic_sb[:, 0:nj], in1=sh, op=OP.subtract
    )
    t1 = gn_pool.tile([8, nj], F32, tag="t1")
    po.tensor_mul(out=t1, in0=y0, in1=y0)
    po.tensor_mul(out=t1, in0=t1, in1=vpe)
    po.tensor_scalar(
        out=t1, in0=t1, scalar1=nrA_sb, scalar2=nrB_sb, op0=OP.mult, op1=OP.add
    )
    po.tensor_mul(out=gs[:, nj:], in0=y0, in1=t1)  # rstd into gs cols nj:
    # expand group stats (mean | rstd) back to per-channel on PE
    pc_ps = psmm.tile(shape, F32, tag=tag, name=f"gnpc_{b}")
    nc.tensor.matmul(
        out=pc_ps[:, 0 : 2 * nj], lhsT=gexp_sb, rhs=gs, start=True, stop=True
    )
    pc = gn_pool.tile([P, 2 * nj], F32, tag="pc")
    if copy_eng == "scalar":
        nc.scalar.activation(out=pc, in_=pc_ps[:, 0 : 2 * nj], func=AF.Copy)
    else:
        nc.vector.tensor_copy(out=pc, in_=pc_ps[:, 0 : 2 * nj])
    jsl = slice(js[0], js[0] + nj)
    po.tensor_mul(out=at[:, jsl], in0=pc[:, nj:], in1=gscale_sb[:, jsl])
    po.tensor_mul(out=bt[:, jsl], in0=pc[:, 0:nj], in1=at[:, jsl])
    po.tensor_sub(out=bt[:, jsl], in0=gbias_sb[:, jsl], in1=bt[:, jsl])


def _kernel_body(nc, tc, ap, out_ap):
    ctx = tc._ctx

    const = ctx.enter_context(tc.tile_pool(name="const", bufs=1))
    gn_pool = ctx.enter_context(tc.tile_pool(name="gn", bufs=2))
    qk_pool = ctx.enter_context(tc.tile_pool(name="qk", bufs=1))
    ew_pool = ctx.enter_context(tc.tile_pool(name="ew", bufs=6))
    rc_pool = ctx.enter_context(tc.tile_pool(name="rc", bufs=3))
    outp = ctx.enter_context(tc.tile_pool(name="outp", bufs=2))
    psA = ctx.enter_context(tc.tile_pool(name="psA", bufs=2, space="PSUM"))
    psB = ctx.enter_context(tc.tile_pool(name="psB", bufs=1, space="PSUM"))
    psC = ctx.enter_context(tc.tile_pool(name="psC", bufs=3, space="PSUM"))
    pools = {"gn": gn_pool, "psA": psA, "psC": psC}

    xv = ap["xr"].rearrange("b (m p) t -> b p m t", p=P)  # residual (pre-biased)
    ov = out_ap.rearrange("b (m p) t -> b m p t", p=P)
    xvr = ap["xbf"].rearrange("b (j p) t -> b p j t", p=P)

    # ------- loads: tiny GN consts first (ACT queue), x + weights on SP -------
    gsel_sb = const.tile([P, 8], F32)
    nc.scalar.dma_start(out=gsel_sb, in_=ap["gsel"])
    gexp_sb = const.tile([8, P], F32)
    nc.scalar.dma_start(out=gexp_sb, in_=ap["gexp"])
    gscale_sb = const.tile([P, 4], F32)
    nc.scalar.dma_start(out=gscale_sb, in_=ap["gscale"])
    gbias_sb = const.tile([P, 4], F32)
    nc.scalar.dma_start(out=gbias_sb, in_=ap["gbias"])
    bqk_sb = const.tile([P, 8], F32)
    nc.scalar.dma_start(out=bqk_sb, in_=ap["bqk"])

    xf = const.tile([P, BL, 4, T], BF16)
    nc.sync.dma_start(out=xf[:, 0, 0:2, :], in_=xvr[0][:, 0:2, :])
    nc.gpsimd.dma_start(out=xf[:, 0, 2:4, :], in_=xvr[0][:, 2:4, :])
    nc.scalar.dma_start(out=xf[:, 1], in_=xvr[1])

    wq_sb = const.tile([P, 4, 3 * C], FP8)  # w_qkv^T * 16: [cin_part, cin_tile, out]
    nc.sync.dma_start(out=wq_sb, in_=ap["wqkvT"].rearrange("(j p) o -> p j o", p=P))
    wp_sb = const.tile([P, 4, C], FP8)  # w_proj^T * 16
    nc.sync.dma_start(out=wp_sb, in_=ap["wprojT"].rearrange("(j p) o -> p j o", p=P))
    eps_sb = const.tile([8, 1], F32)
    nc.vector.memset(eps_sb, EPS)
    ebias_sb = const.tile([P, 1], F32)
    nc.vector.memset(ebias_sb, EXP_BIAS)
    magic_sb = const.tile([8, 4], U32)
    nc.vector.memset(magic_sb, RSQRT_MAGIC)
    nrA_sb = const.tile([8, 1], F32)
    nc.vector.memset(nrA_sb, -0.5)
    nrB_sb = const.tile([8, 1], F32)
    nc.vector.memset(nrB_sb, 1.5)
    consts = (gscale_sb, gbias_sb, gsel_sb, gexp_sb, eps_sb, magic_sb, nrA_sb, nrB_sb)

    # persistent data tiles
    xh = const.tile([P, BL, 4, T], FP8)  # normalized h
    q_sb = qk_pool.tile([P, BL, 4, T], BF16, tag="q")
    k_sb = qk_pool.tile([P, BL, 4, T], BF16, tag="k")
    # V^T, s-tile-pair major for DoubleRow AV: [p, b, s2, i, (h w)]
    vt2 = qk_pool.tile([P, BL, 4, 2, NH * VT_W], FP8, tag="vt")
    a_sb = qk_pool.tile([P, BL, 4, T], FP8, tag="a")
    at_t = [const.tile([P, 4], F32, name=f"at{b}") for b in range(BL)]
    bt_t = [const.tile([P, 4], F32, name=f"bt{b}") for b in range(BL)]

    # ones columns of V^T (softmax denominators ride the AV matmul)
    for b in range(BL):
        for s2 in range(4):
            for i in range(2):
                ones_view = vt2[:, b, s2, i, :].rearrange(
                    "p (h w) -> p h w", w=VT_W
                )[:, :, CH : CH + 1]
                nc.vector.memset(ones_view, 1.0)

    # ---------------- per-batch building blocks ----------------
    def gn_stats(b, js):
        return _gn_phase1(nc, tc, pools, xf, b, js, consts)

    def gn_finish(b, js, m2, stream=False, chain_eng="pool"):
        p = {**pools, "psu": psB} if stream else pools
        _gn_phase2(
            nc, tc, p, b, js, m2, at_t[b], bt_t[b], consts,
            chain_eng=chain_eng, copy_eng="vector" if stream else "scalar",
        )

    def xh_evict(b, j, engine="vector"):
        if engine == "scalar":
            # ACT Identity: free during startup, same act table as Exp
            nc.scalar.activation(
                out=xh[:, b, j, :],
                in_=xf[:, b, j, :],
                func=AF.Identity,
                bias=bt_t[b][:, j : j + 1],
                scale=at_t[b][:, j : j + 1],
            )
        else:
            eng = nc.gpsimd if engine == "pool" else nc.vector
            eng.tensor_scalar(
                out=xh[:, b, j, :],
                in0=xf[:, b, j, :],
                scalar1=at_t[b][:, j : j + 1],
                scalar2=bt_t[b][:, j : j + 1],
                op0=OP.mult,
                op1=OP.add,
            )

    def qk_tile(b, m, engine="vector", stream=False):
        """Q (m<4) or K (m>=4) output tile m: fp8 DoubleRow over cin pairs.

        Prologue tiles use the wide psA rotation; stream units use the
        single-bank psB pool per half so their evictions never gate the
        attention S-tile rotation."""
        dst = q_sb[:, b, m, :] if m < 4 else k_sb[:, b, m - 4, :]
        ps_full = None if stream else psA.tile([P, T], F32, tag="mm")
        for n in range(2):
            nsl = slice(512 * n, 512 * (n + 1))
            ps = (
                psB.tile([P, 512], F32, tag="u", name="qkps")
                if stream
                else ps_full[:, nsl]
            )
            for jp in range(2):
                nc.tensor.matmul(
                    out=ps,
                    lhsT=wq_sb[:, 2 * jp : 2 * jp + 2, P * m : P * (m + 1)],
                    rhs=xh[:, b, 2 * jp : 2 * jp + 2, nsl],
                    start=(jp == 0),
                    stop=(jp == 1),
                    perf_mode=DR,
                )
            if not stream and n == 0:
                continue  # prologue: single eviction over the full tile
            src = ps if stream else ps_full
            osl = nsl if stream else slice(0, T)
            if engine == "scalar":
                nc.scalar.activation(
                    out=dst[:, osl],
                    in_=src,
                    func=AF.Identity,
                    bias=bqk_sb[:, m : m + 1],
                    scale=1.0 / WSCALE,
                )
            else:
                nc.vector.tensor_scalar(
                    out=dst[:, osl],
                    in0=src,
                    scalar1=1.0 / WSCALE,
                    scalar2=bqk_sb[:, m : m + 1],
                    op0=OP.mult,
                    op1=OP.add,
                )

    def v_tile(b, s, stream=False):
        """V^T s-block: [s 128, c 512] via DoubleRow, evict *1/16 to fp8."""
        if stream:
            ps = psB.tile([P, 512], F32, tag="u")
        else:
            ps_w = psA.tile([P, T], F32, tag="mm", name="vps")
            ps = ps_w[:, 0:C]
        for jp in range(2):
            nc.tensor.matmul(
                out=ps,
                lhsT=xh[:, b, 2 * jp : 2 * jp + 2, P * s : P * (s + 1)],
                rhs=wq_sb[:, 2 * jp : 2 * jp + 2, 2 * C : 3 * C],
                start=(jp == 0),
                stop=(jp == 1),
                perf_mode=DR,
            )
        dst = vt2[:, b, s // 2, s % 2, :].rearrange("p (h w) -> p h w", w=VT_W)[
            :, :, 0:CH
        ]
        nc.vector.tensor_scalar_mul(
            out=dst,
            in0=ps.rearrange("p (h c) -> p h c", c=CH),
            scalar1=1.0 / WSCALE,
        )

    def qk_pair(b, h, s2):
        """S^T for s-tiles (2*s2, 2*s2+1) -> exp -> paired fp8 ew tile."""
        jt, pof = h // 2, CH * (h % 2)
        qh = q_sb[pof : pof + CH, b, jt, :]
        kh = k_sb[pof : pof + CH, b, jt, :]
        ew = ew_pool.tile([P, 2, T], FP8, tag="ew")
        for i in range(2):
            s = 2 * s2 + i
            sps = psA.tile([P, T], F32, tag="mm")
            for n in range(2):
                nc.tensor.matmul(
                    out=sps[:, 512 * n : 512 * (n + 1)],
                    lhsT=kh[:, P * s : P * (s + 1)],
                    rhs=qh[:, 512 * n : 512 * (n + 1)],
                    start=True,
                    stop=True,
                )
            nc.scalar.activation(out=ew[:, i, :], in_=sps, func=AF.Exp, bias=ebias_sb)
        return ew

    def av_pair(b, h, s2, ew, accs):
        for n in range(2):
            nc.tensor.matmul(
                out=accs[n],
                lhsT=vt2[:, b, s2, :, VT_W * h : VT_W * h + VT_USED],
                rhs=ew[:, :, 512 * n : 512 * (n + 1)],
                start=(s2 == 0),
                stop=(s2 == 3),
                perf_mode=DR,
            )

    def normalize(b, h, accs):
        """softmax denominators live in row 64 of each acc half.

        DVE supports shifted partition bases (verified on HW): reciprocal
        reads acc partition 64 and writes partition 0 directly, and the
        normalize multiply writes partitions 64-127 for odd heads."""
        jt, pof = h // 2, CH * (h % 2)
        for n in range(2):
            acc = accs[n]
            nsl = slice(512 * n, 512 * (n + 1))
            rz = rc_pool.tile([1, 512], F32, tag="rz")
            nc.vector.reciprocal(out=rz, in_=acc[CH : CH + 1, :])
            rb = rc_pool.tile([CH, 512], F32, tag="rb")
            nc.gpsimd.partition_broadcast(out_ap=rb, in_ap=rz, channels=CH)
            nc.vector.tensor_mul(
                out=a_sb[pof : pof + CH, b, jt, nsl], in0=acc[0:CH, :], in1=rb
            )

    def proj_tile(b, m, xr_t, store_engines, act_assist=False):
        pps = psA.tile([P, T], F32, tag="mm")
        o_t = outp.tile([P, T], F32, tag="o")
        for n in range(2):
            nsl = slice(512 * n, 512 * (n + 1))
            for jp in range(2):
                nc.tensor.matmul(
                    out=pps[:, nsl],
                    lhsT=wp_sb[:, 2 * jp : 2 * jp + 2, P * m : P * (m + 1)],
                    rhs=a_sb[:, b, 2 * jp : 2 * jp + 2, nsl],
                    start=(jp == 0),
                    stop=(jp == 1),
                    perf_mode=DR,
                )
            # per-half eviction + store so the tail pipelines
            if act_assist and n == 0:
                # tail only (ACT idle after its last exp): evict on ACT,
                # residual add on Pool
                tmp = outp.tile([P, 512], F32, tag="otmp")
                nc.scalar.activation(
                    out=tmp, in_=pps[:, nsl], func=AF.Identity, scale=1.0 / WSCALE
                )
                nc.gpsimd.tensor_add(out=o_t[:, nsl], in0=tmp, in1=xr_t[:, m, nsl])
            else:
                nc.vector.scalar_tensor_tensor(
                    out=o_t[:, nsl],
                    in0=pps[:, nsl],
                    scalar=1.0 / WSCALE,
                    in1=xr_t[:, m, nsl],
                    op0=OP.mult,
                    op1=OP.add,
                )
            store_engines[n].dma_start(out=ov[b, m][:, nsl], in_=o_t[:, nsl])

    # ---------------- emission schedule ----------------
    # Startup critical chain: xf(b0) -> GN stats -> xh (ACT Identity) ->
    # QK m0/m4 (ACT Identity evicts) -> first softmax exp.  DVE meanwhile
    # works through the b0 V / remaining Q,K evictions in deadline order;
    # b1's GN finisher, xh (Pool) and qkv run as stream units.
    M_ORDER = [0, 4, 1, 5, 2, 6, 3, 7]  # Q/K tile order: head h needs (h//2, 4+h//2)

    m2_a = gn_stats(0, [0, 1])
    gn_finish(0, [0, 1], m2_a, chain_eng="pool")
    xh_evict(0, 0, "scalar")
    xh_evict(0, 1, "vector")
    m2_b = gn_stats(0, [2, 3])
    gn_finish(0, [2, 3], m2_b, chain_eng="pool")
    xh_evict(0, 2, "pool")
    xh_evict(0, 3, "scalar")
    qk_tile(0, 0, "scalar")
    qk_tile(0, 4, "vector")
    for s in range(3):
        v_tile(0, s, stream=True)  # psB: keeps the psA S rotation clean

    xr0 = outp.tile([P, 4, T], F32, tag="xr", bufs=1)
    xr1 = outp.tile([P, 4, T], F32, tag="xr1", bufs=1)

    # Flat attention pipeline over all (batch, head) pairs.  AV matmuls lag
    # the QK/exp stream by AV_LAG pairs (PE is in-order: an AV waiting on its
    # exp would block the next QK matmul and starve ACT).  Interleaved units
    # are emitted BEFORE the lagged AV/normalize so their PSUM evictions sit
    # ahead of the normalize ops in the DVE queue.
    heads = [(0, h) for h in range(NH)] + [(1, h) for h in range(NH)]
    AV_LAG = 3
    NPAIR = 4 * len(heads)
    pending = {}  # pair index -> (b, h, s2, ew)
    accs_of = {}  # head index -> acc tiles

    units = {}
    units[0] = lambda: qk_tile(0, 1, stream=True)
    units[1] = lambda: qk_tile(0, 5, stream=True)
    for s in range(3, 8):  # v(0) s3..s7: evicted just ahead of their AV pair
        units[s - 1] = lambda s=s: v_tile(0, s, stream=True)
    units[7] = lambda: qk_tile(0, 2, stream=True)
    units[8] = lambda: qk_tile(0, 6, stream=True)
    units[9] = lambda: qk_tile(0, 3, stream=True)
    units[10] = lambda: qk_tile(0, 7, stream=True)
    # b1 GroupNorm: stats split into 1-tile units, finisher; xh on Pool
    gn1_state = {}
    for u, js in enumerate(([0], [1], [2], [3])):
        units[11 + u] = lambda js=js: gn1_state.setdefault(
            js[0], gn_stats(1, js)
        )
    units[15] = lambda: gn_finish(1, [0, 1, 2, 3], _merge_m2(gn1_state), stream=True)
    units[16] = lambda: tuple(xh_evict(1, j, "pool") for j in range(4))
    for j, (kind, idx) in enumerate(
        [("qk", m) for m in M_ORDER] + [("v", s) for s in range(8)]
    ):
        units[17 + j] = lambda k=kind, i=idx: (
            qk_tile(1, i, stream=True) if k == "qk" else v_tile(1, i, stream=True)
        )
    units[33] = lambda: nc.sync.dma_start(out=xr0, in_=xv[0])
    units[59] = lambda: nc.sync.dma_start(out=xr1, in_=xv[1])

    def proj_half(b, m, n, state, xr_t, store_engine):
        if n == 0:
            state[m] = outp.tile([P, T], F32, tag="o", name=f"o_{b}_{m}")
        o_t = state[m]
        pps = psB.tile([P, 512], F32, tag="u", name=f"pps_{b}_{m}_{n}")
        nsl = slice(512 * n, 512 * (n + 1))
        for jp in range(2):
            nc.tensor.matmul(
                out=pps,
                lhsT=wp_sb[:, 2 * jp : 2 * jp + 2, P * m : P * (m + 1)],
                rhs=a_sb[:, b, 2 * jp : 2 * jp + 2, nsl],
                start=(jp == 0),
                stop=(jp == 1),
                perf_mode=DR,
            )
        nc.vector.scalar_tensor_tensor(
            out=o_t[:, nsl],
            in0=pps,
            scalar=1.0 / WSCALE,
            in1=xr_t[:, m, nsl],
            op0=OP.mult,
            op1=OP.add,
        )
        store_engine.dma_start(out=ov[b, m][:, nsl], in_=o_t[:, nsl])

    proj0_state = {}
    for j in range(4):
        units[40 + 5 * j] = lambda m=j: proj_half(0, m, 0, proj0_state, xr0, nc.sync)
        units[41 + 5 * j] = lambda m=j: proj_half(0, m, 1, proj0_state, xr0, nc.sync)

    def _merge_m2(state):
        # gn_stats on [j] writes a [P,2] m2 each; build the [P,8] layout
        # _gn_phase2 expects: cols 0:4 means, 4:8 E[x^2]
        m2 = gn_pool.tile([P, 8], F32, tag="m2m", name="m2_merged")
        for j in range(4):
            nc.vector.tensor_copy(out=m2[:, j : j + 1], in_=state[j][:, 0:1])
            nc.vector.tensor_copy(out=m2[:, 4 + j : 5 + j], in_=state[j][:, 1:2])
        return m2

    def emit_av(g):
        b, h, s2, ew = pending.pop(g)
        if s2 == 0:
            accs_of[g // 4] = [
                psC.tile([VT_USED, 512], F32, tag="av", name=f"acc_{b}_{h}_{n}")
                for n in range(2)
            ]
        av_pair(b, h, s2, ew, accs_of[g // 4])
        if s2 == 3:
            normalize(b, h, accs_of.pop(g // 4))

    for g in range(NPAIR):
        hb, s2 = heads[g // 4], g % 4
        pending[g] = (*hb, s2, qk_pair(*hb, s2))
        if g in units:
            units[g]()
        if g >= AV_LAG:
            emit_av(g - AV_LAG)
    for g in range(NPAIR - AV_LAG, NPAIR):
        emit_av(g)

    for m in range(4):
        proj_tile(1, m, xr1, [nc.scalar, nc.sync], act_assist=True)

    if "dbg_xh" in ap:
        nc.sync.dma_start(out=ap["dbg_xh"].rearrange("b (j p) t -> p b j t", p=P), in_=xh)
        nc.sync.dma_start(out=ap["dbg_q"].rearrange("b (j p) t -> p b j t", p=P), in_=q_sb)
        nc.sync.dma_start(out=ap["dbg_k"].rearrange("b (j p) t -> p b j t", p=P), in_=k_sb)
        nc.sync.dma_start(
            out=ap["dbg_vt"].rearrange("b s2 i (p w) -> p b s2 i w", p=P), in_=vt2
        )
        nc.sync.dma_start(out=ap["dbg_a"].rearrange("b (j p) t -> p b j t", p=P), in_=a_sb)


def build(num_devices=NCORES, debug=False, debug_taps=False):
    from concourse import bacc

    nc = bacc.Bacc(
        "TRN2", target_bir_lowering=False, debug=debug, num_devices=num_devices
    )
    ap = {}

    def inp(name, shape, dt=F32):
        ap[name] = nc.dram_tensor(name, shape, dt, kind="ExternalInput").ap()

    inp("xr", [BL, C, T])
    inp("xbf", [BL, C, T], BF16)
    inp("wqkvT", [C, 3 * C], FP8)
    inp("wprojT", [C, C], FP8)
    inp("bqk", [P, 8])
    inp("gscale", [P, 4])
    inp("gbias", [P, 4])
    inp("gsel", [P, 8])
    inp("gexp", [8, P])
    out_ap = nc.dram_tensor("out", [BL, C, T], F32, kind="ExternalOutput").ap()
    if debug_taps:
        for nm, shape, dt in [
            ("dbg_xh", [BL, C, T], FP8),
            ("dbg_q", [BL, C, T], BF16),
            ("dbg_k", [BL, C, T], BF16),
            ("dbg_vt", [BL, 4, 2, P * NH * VT_W], FP8),
            ("dbg_a", [BL, C, T], FP8),
        ]:
            ap[nm] = nc.dram_tensor(nm, shape, dt, kind="ExternalOutput").ap()

    with tile.TileContext(nc) as tc:
        with ExitStack() as ctx:
            tc._ctx = ctx
            _kernel_body(nc, tc, ap, out_ap)
    nc.compile()
    return nc


def host_prep(x, gn_scale, gn_bias, w_qkv, b_qkv, w_proj, b_proj):
    """Shared (weight) arrays + per-batch residual/bf16 x arrays."""
    import ml_dtypes

    xr = np.ascontiguousarray(np.asarray(x, np.float32).reshape(B, C, T))
    w_qkv = np.asarray(w_qkv, np.float32)
    b_qkv = np.asarray(b_qkv, np.float32)
    w_proj = np.asarray(w_proj, np.float32)
    b_proj = np.asarray(b_proj, np.float32)
    # permute interleaved [head, (q,k,v), ch] rows -> [(q,k,v), head, ch]
    perm = np.array(
        [h * 3 * CH + w * CH + c for w in range(3) for h in range(NH) for c in range(CH)],
        dtype=np.int64,
    )
    wq_p = w_qkv[perm].copy()
    bq_p = b_qkv[perm].copy()
    wq_p[:C] *= 0.125  # attention scale (1/sqrt(sqrt(ch)))^2 folded into Q
    bq_p[:C] *= 0.125
    bv = bq_p[2 * C :]  # V bias: folded into the residual via W_p @ bv
    # residual pre-bias: out = proj(a) + (x + b_proj + W_p @ bv)
    resid_bias = b_proj + w_proj @ bv
    xresid = xr + resid_bias[None, :, None].astype(np.float32)

    shared = {
        "wqkvT": np.ascontiguousarray((wq_p * WSCALE).T).astype(
            ml_dtypes.float8_e4m3
        ),
        "wprojT": np.ascontiguousarray((w_proj * WSCALE).T).astype(
            ml_dtypes.float8_e4m3
        ),
        "bqk": np.ascontiguousarray(bq_p[: 2 * C].reshape(8, P).T),
        "gscale": np.ascontiguousarray(gn_scale.reshape(4, P).T.astype(np.float32)),
        "gbias": np.ascontiguousarray(gn_bias.reshape(4, P).T.astype(np.float32)),
        "gsel": np.ascontiguousarray(
            (np.arange(P)[:, None] // GS == np.arange(8)[None, :]).astype(np.float32)
            / GS
        ),
        "gexp": np.ascontiguousarray(
            (np.arange(8)[:, None] == np.arange(P)[None, :] // GS).astype(np.float32)
        ),
    }
    return xr, xresid, shared


_NC_CACHE = {}


def make_in_maps(inputs):
    import ml_dtypes

    xr, xresid, shared = host_prep(**inputs)
    xbf = xr.astype(ml_dtypes.bfloat16)
    return [
        {
            "xr": np.ascontiguousarray(xresid[i * BL : (i + 1) * BL]),
            "xbf": np.ascontiguousarray(xbf[i * BL : (i + 1) * BL]),
            **shared,
        }
        for i in range(NCORES)
    ]


def kernel(x, gn_scale, gn_bias, w_qkv, b_qkv, w_proj, b_proj):
    in_maps = make_in_maps(
        dict(
            x=x,
            gn_scale=gn_scale,
            gn_bias=gn_bias,
            w_qkv=w_qkv,
            b_qkv=b_qkv,
            w_proj=w_proj,
            b_proj=b_proj,
        )
    )
    if "nc" not in _NC_CACHE:
        _NC_CACHE["nc"] = build()
    nc = _NC_CACHE["nc"]
    res = run_bass_kernel_spmd(nc, in_maps, core_ids=list(range(NCORES)))
    out = np.concatenate([res.results[i]["out"] for i in range(NCORES)], axis=0)
    return np.ascontiguousarray(out.reshape(B, C, 32, 32).astype(np.float32))


# revision 7
# speedup vs baseline: 1.0006x; 1.0006x over previous
"""AttentionBlock (GroupNorm + 8-head attention + proj + residual) on 8 TRN2 NeuronCores.

ACT-bound pipeline design. Data-parallel over batch (2 per core, no
collectives). The per-core floor is the softmax exp stream on the scalar
(ACT) engine: 16 (batch,head) x T^2 = 16.8M exps = 128 x [128,1024]
activations ~ 133us; everything else is scheduled to hide under it.

  - ACT runs ONLY Exp (+ a few Identity/Copy ops during the idle startup
    window; all share one act table -> a single table load).  GroupNorm
    rstd avoids ACT Sqrt via a bit-hack rsqrt + Newton step on Pool/DVE.
  - All steady-state PSUM evictions run on DVE (tensor_scalar with
    per-partition AP scalars); Pool takes the GN small-op chains, the
    xh(b1) eviction, softmax-reciprocal broadcasts, and tail residual adds.
  - fp8e4 DoubleRow matmuls (0.5 cyc/row, 2x contraction per instr) for
    qkv, AV and proj; QK^T stays bf16.  DoubleRow weight slices need
    16B-aligned strides -> V^T pads each head to VT_W=66 columns.
  - exp outputs fp8 with a -2.5 logit bias (e4m3 range); softmax ratios are
    bias-invariant.  A ones-column in V^T makes the AV matmul accumulate
    softmax denominators for free; normalization uses partition-shifted DVE
    reciprocal/multiply (verified on HW) -> no SBUF-shuffle DMAs.
  - V bias and proj bias fold host-side: softmax weights sum to 1, so
    out = proj(sum w v) + (x + bp + Wp bv); the residual is pre-biased.
  - PSUM: 2x[128,1024] banks are reserved for the attention S-tiles; all
    interleaved work (qkv(b1), proj(b0), GN) evicts from a separate
    single-bank pool so it never gates the S rotation; AV accumulators
    rotate through 3 single-bank tiles.
  - Emission is one flat software-pipelined stream: 64 QK pairs with AV
    lagging 3 pairs, interleaved units (b1 GN/qkv, proj(b0), residual
    loads) placed by deadline, per-half proj tail on ACT+Pool/DVE.
"""

import numpy as np
from contextlib import ExitStack

import concourse.bass as bass
import concourse.tile as tile
from concourse import mybir
from concourse.bass_utils import run_bass_kernel_spmd

B, C, T = 16, 512, 1024
NH, CH = 8, 64
GS = 16  # channels per GroupNorm group
EPS = 1e-5
NCORES = 8
BL = B // NCORES  # batches per core
P = 128
F32 = mybir.dt.float32
BF16 = mybir.dt.bfloat16
FP8 = mybir.dt.float8e4
AF = mybir.ActivationFunctionType
OP = mybir.AluOpType
DR = mybir.MatmulPerfMode.DoubleRow

VT_W = 66  # per-head V^T columns: 64 ch + 1 ones col + 1 pad so the
# DoubleRow s-pair stride (NH*VT_W fp8 bytes) is 16B-aligned (HW requirement)
VT_USED = 65  # columns actually consumed by the AV matmul
WSCALE = 16.0  # fp8 weight scale (folded back out at PSUM eviction)
EXP_BIAS = -2.5  # logit shift for fp8 exp range; softmax-invariant


U32 = mybir.dt.uint32
RSQRT_MAGIC = 0x5F3759DF


def _gn_phase1(nc, tc, pools, xf, b, js, consts):
    """DVE-only GroupNorm stats for batch b, c-tiles `js`: returns the
    per-channel (mean | E[x^2]) tile m2."""
    gn_pool = pools["gn"]
    nj = len(js)
    bnraw = gn_pool.tile([P, nj, 2, 6], F32, tag="bnraw")
    mv = gn_pool.tile([P, nj, 2], F32, tag="mv")
    for ji, j in enumerate(js):
        for hf in range(2):
            nc.vector.bn_stats(
                out=bnraw[:, ji, hf, :], in_=xf[:, b, j, 512 * hf : 512 * (hf + 1)]
            )
        nc.vector.bn_aggr(out=mv[:, ji, :], in_=bnraw[:, ji, :, :])
    # m2: cols 0:nj per-channel mean (per c-tile), nj:2nj per-channel E[x^2]
    m2 = gn_pool.tile([P, 2 * nj], F32, tag="m2")
    nc.vector.tensor_copy(out=m2[:, 0:nj], in_=mv[:, :, 0])
    nc.vector.tensor_mul(out=m2[:, nj:], in0=mv[:, :, 0], in1=mv[:, :, 0])
    nc.vector.tensor_add(out=m2[:, nj:], in0=m2[:, nj:], in1=mv[:, :, 1])
    return m2


def _gn_phase2(
    nc, tc, pools, b, js, m2, at, bt, consts, chain_eng="pool", copy_eng="vector"
):
    """Group aggregation (PE) + rstd via bit-hack rsqrt + 1 Newton step.
    The serial small-op chain runs on Pool or DVE (chain_eng) so the two
    b0 half-chains execute in parallel.  ACT stays exp-only."""
    gn_pool, psA = pools["gn"], pools["psA"]
    gscale_sb, gbias_sb, gsel_sb, gexp_sb, eps_sb, magic_sb, nrA_sb, nrB_sb = consts
    nj = len(js)
    po = nc.gpsimd if chain_eng == "pool" else nc.vector

    # group-aggregate across the 16-channel groups (partition dim) on PE;
    # gsel carries the 1/16 group mean scaling
    psmm = pools.get("psu") or psA
    shape = [P, 512] if "psu" in pools else [P, T]
    tag = "u" if "psu" in pools else "mm"
    gst_ps = psmm.tile(shape, F32, tag=tag, name=f"gnst_{b}")
    nc.tensor.matmul(
        out=gst_ps[0:8, 0 : 2 * nj], lhsT=gsel_sb, rhs=m2, start=True, stop=True
    )
    gs = gn_pool.tile([8, 2 * nj], F32, tag="gs")  # cols 0:nj mu_g, nj: E2_g
    if copy_eng == "scalar":
        nc.scalar.activation(out=gs, in_=gst_ps[0:8, 0 : 2 * nj], func=AF.Copy)
    else:
        nc.vector.tensor_copy(out=gs, in_=gst_ps[0:8, 0 : 2 * nj])
    musq = gn_pool.tile([8, nj], F32, tag="musq")
    po.tensor_mul(out=musq, in0=gs[:, 0:nj], in1=gs[:, 0:nj])
    vpe = gn_pool.tile([8, nj], F32, tag="vpe")
    po.tensor_sub(out=vpe, in0=gs[:, nj:], in1=musq)
    po.tensor_scalar_add(out=vpe, in0=vpe, scalar1=eps_sb)
    # rstd = rsqrt(vpe): exponent bit-hack seed + 1 Newton-Raphson step.
    # Immediate-scalar tensor_scalar is not supported on Pool -> DVE.
    sh = gn_pool.tile([8, nj], U32, tag="sh")
    nc.vector.tensor_scalar(
        out=sh, in0=vpe.bitcast(U32), scalar1=1, scalar2=None, op0=OP.arith_shift_right
    )
    y0 = gn_pool.tile([8, nj], F32, tag="y0")
    po.tensor_tensor(
        out=y0.bitcast(U32), in0=magic_sb[:, 0:nj], in1=sh, op=OP.subtract
    )
    t1 = gn_pool.tile([8, nj], F32, tag="t1")
    po.tensor_mul(out=t1, in0=y0, in1=y0)
    po.tensor_mul(out=t1, in0=t1, in1=vpe)
    po.tensor_scalar(
        out=t1, in0=t1, scalar1=nrA_sb, scalar2=nrB_sb, op0=OP.mult, op1=OP.add
    )
    po.tensor_mul(out=gs[:, nj:], in0=y0, in1=t1)  # rstd into gs cols nj:
    # expand group stats (mean | rstd) back to per-channel on PE
    pc_ps = psmm.tile(shape, F32, tag=tag, name=f"gnpc_{b}")
    nc.tensor.matmul(
        out=pc_ps[:, 0 : 2 * nj], lhsT=gexp_sb, rhs=gs, start=True, stop=True
    )
    pc = gn_pool.tile([P, 2 * nj], F32, tag="pc")
    if copy_eng == "scalar":
        nc.scalar.activation(out=pc, in_=pc_ps[:, 0 : 2 * nj], func=AF.Copy)
    else:
        nc.vector.tensor_copy(out=pc, in_=pc_ps[:, 0 : 2 * nj])
    jsl = slice(js[0], js[0] + nj)
    po.tensor_mul(out=at[:, jsl], in0=pc[:, nj:], in1=gscale_sb[:, jsl])
    po.tensor_mul(out=bt[:, jsl], in0=pc[:, 0:nj], in1=at[:, jsl])
    po.tensor_sub(out=bt[:, jsl], in0=gbias_sb[:, jsl], in1=bt[:, jsl])


def _kernel_body(nc, tc, ap, out_ap):
    ctx = tc._ctx

    const = ctx.enter_context(tc.tile_pool(name="const", bufs=1))
    gn_pool = ctx.enter_context(tc.tile_pool(name="gn", bufs=2))
    qk_pool = ctx.enter_context(tc.tile_pool(name="qk", bufs=1))
    ew_pool = ctx.enter_context(tc.tile_pool(name="ew", bufs=6))
    rc_pool = ctx.enter_context(tc.tile_pool(name="rc", bufs=3))
    outp = ctx.enter_context(tc.tile_pool(name="outp", bufs=2))
    psA = ctx.enter_context(tc.tile_pool(name="psA", bufs=2, space="PSUM"))
    psB = ctx.enter_context(tc.tile_pool(name="psB", bufs=1, space="PSUM"))
    psC = ctx.enter_context(tc.tile_pool(name="psC", bufs=3, space="PSUM"))
    pools = {"gn": gn_pool, "psA": psA, "psC": psC}

    xv = ap["xr"].rearrange("b (m p) t -> b p m t", p=P)  # residual (pre-biased)
    ov = out_ap.rearrange("b (m p) t -> b m p t", p=P)
    xvr = ap["xbf"].rearrange("b (j p) t -> b p j t", p=P)

    # ------- loads: tiny GN consts first (ACT queue), x + weights on SP -------
    gsel_sb = const.tile([P, 8], F32)
    nc.scalar.dma_start(out=gsel_sb, in_=ap["gsel"])
    gexp_sb = const.tile([8, P], F32)
    nc.scalar.dma_start(out=gexp_sb, in_=ap["gexp"])
    gscale_sb = const.tile([P, 4], F32)
    nc.scalar.dma_start(out=gscale_sb, in_=ap["gscale"])
    gbias_sb = const.tile([P, 4], F32)
    nc.scalar.dma_start(out=gbias_sb, in_=ap["gbias"])
    bqk_sb = const.tile([P, 8], F32)
    nc.scalar.dma_start(out=bqk_sb, in_=ap["bqk"])

    xf = const.tile([P, BL, 4, T], BF16)
    nc.sync.dma_start(out=xf[:, 0, 0:2, :], in_=xvr[0][:, 0:2, :])
    nc.gpsimd.dma_start(out=xf[:, 0, 2:4, :], in_=xvr[0][:, 2:4, :])
    nc.scalar.dma_start(out=xf[:, 1], in_=xvr[1])

    wq_sb = const.tile([P, 4, 3 * C], FP8)  # w_qkv^T * 16: [cin_part, cin_tile, out]
    nc.sync.dma_start(out=wq_sb, in_=ap["wqkvT"].rearrange("(j p) o -> p j o", p=P))
    wp_sb = const.tile([P, 4, C], FP8)  # w_proj^T * 16
    nc.sync.dma_start(out=wp_sb, in_=ap["wprojT"].rearrange("(j p) o -> p j o", p=P))
    eps_sb = const.tile([8, 1], F32)
    nc.vector.memset(eps_sb, EPS)
    ebias_sb = const.tile([P, 1], F32)
    nc.vector.memset(ebias_sb, EXP_BIAS)
    magic_sb = const.tile([8, 4], U32)
    nc.vector.memset(magic_sb, RSQRT_MAGIC)
    nrA_sb = const.tile([8, 1], F32)
    nc.vector.memset(nrA_sb, -0.5)
    nrB_sb = const.tile([8, 1], F32)
    nc.vector.memset(nrB_sb, 1.5)
    consts = (gscale_sb, gbias_sb, gsel_sb, gexp_sb, eps_sb, magic_sb, nrA_sb, nrB_sb)

    # persistent data tiles
    xh = const.tile([P, BL, 4, T], FP8)  # normalized h
    q_sb = qk_pool.tile([P, BL, 4, T], BF16, tag="q")
    k_sb = qk_pool.tile([P, BL, 4, T], BF16, tag="k")
    # V^T, s-tile-pair major for DoubleRow AV: [p, b, s2, i, (h w)]
    vt2 = qk_pool.tile([P, BL, 4, 2, NH * VT_W], FP8, tag="vt")
    a_sb = qk_pool.tile([P, BL, 4, T], FP8, tag="a")
    at_t = [const.tile([P, 4], F32, name=f"at{b}") for b in range(BL)]
    bt_t = [const.tile([P, 4], F32, name=f"bt{b}") for b in range(BL)]

    # ones columns of V^T (softmax denominators ride the AV matmul)
    for b in range(BL):
        for s2 in range(4):
            for i in range(2):
                ones_view = vt2[:, b, s2, i, :].rearrange(
                    "p (h w) -> p h w", w=VT_W
                )[:, :, CH : CH + 1]
                nc.vector.memset(ones_view, 1.0)

    # ---------------- per-batch building blocks ----------------
    def gn_stats(b, js):
        return _gn_phase1(nc, tc, pools, xf, b, js, consts)

    def gn_finish(b, js, m2, stream=False, chain_eng="pool"):
        p = {**pools, "psu": psB} if stream else pools
        _gn_phase2(
            nc, tc, p, b, js, m2, at_t[b], bt_t[b], consts,
            chain_eng=chain_eng, copy_eng="vector" if stream else "scalar",
        )

    def xh_evict(b, j, engine="vector"):
        if engine == "scalar":
            # ACT Identity: free during startup, same act table as Exp
            nc.scalar.activation(
                out=xh[:, b, j, :],
                in_=xf[:, b, j, :],
                func=AF.Identity,
                bias=bt_t[b][:, j : j + 1],
                scale=at_t[b][:, j : j + 1],
            )
        else:
            eng = nc.gpsimd if engine == "pool" else nc.vector
            eng.tensor_scalar(
                out=xh[:, b, j, :],
                in0=xf[:, b, j, :],
                scalar1=at_t[b][:, j : j + 1],
                scalar2=bt_t[b][:, j : j + 1],
                op0=OP.mult,
                op1=OP.add,
            )

    def qk_tile(b, m, engine="vector", stream=False):
        """Q (m<4) or K (m>=4) output tile m: fp8 DoubleRow over cin pairs.

        Prologue tiles use the wide psA rotation; stream units use the
        single-bank psB pool per half so their evictions never gate the
        attention S-tile rotation."""
        dst = q_sb[:, b, m, :] if m < 4 else k_sb[:, b, m - 4, :]
        ps_full = None if stream else psA.tile([P, T], F32, tag="mm")
        for n in range(2):
            nsl = slice(512 * n, 512 * (n + 1))
            ps = (
                psB.tile([P, 512], F32, tag="u", name="qkps")
                if stream
                else ps_full[:, nsl]
            )
            for jp in range(2):
                nc.tensor.matmul(
                    out=ps,
                    lhsT=wq_sb[:, 2 * jp : 2 * jp + 2, P * m : P * (m + 1)],
                    rhs=xh[:, b, 2 * jp : 2 * jp + 2, nsl],
                    start=(jp == 0),
                    stop=(jp == 1),
                    perf_mode=DR,
                )
            # evict per half everywhere: lets the first QK matmul start on
            # the n=0 half while n=1 is still evicting
            src = ps if stream else ps_full[:, nsl]
            osl = nsl
            if engine == "scalar":
                nc.scalar.activation(
                    out=dst[:, osl],
                    in_=src,
                    func=AF.Identity,
                    bias=bqk_sb[:, m : m + 1],
                    scale=1.0 / WSCALE,
                )
            else:
                nc.vector.tensor_scalar(
                    out=dst[:, osl],
                    in0=src,
                    scalar1=1.0 / WSCALE,
                    scalar2=bqk_sb[:, m : m + 1],
                    op0=OP.mult,
                    op1=OP.add,
                )

    def v_tile(b, s, stream=False):
        """V^T s-block: [s 128, c 512] via DoubleRow, evict *1/16 to fp8."""
        if stream:
            ps = psB.tile([P, 512], F32, tag="u")
        else:
            ps_w = psA.tile([P, T], F32, tag="mm", name="vps")
            ps = ps_w[:, 0:C]
        for jp in range(2):
            nc.tensor.matmul(
                out=ps,
                lhsT=xh[:, b, 2 * jp : 2 * jp + 2, P * s : P * (s + 1)],
                rhs=wq_sb[:, 2 * jp : 2 * jp + 2, 2 * C : 3 * C],
                start=(jp == 0),
                stop=(jp == 1),
                perf_mode=DR,
            )
        dst = vt2[:, b, s // 2, s % 2, :].rearrange("p (h w) -> p h w", w=VT_W)[
            :, :, 0:CH
        ]
        nc.vector.tensor_scalar_mul(
            out=dst,
            in0=ps.rearrange("p (h c) -> p h c", c=CH),
            scalar1=1.0 / WSCALE,
        )

    def qk_pair(b, h, s2):
        """S^T for s-tiles (2*s2, 2*s2+1) -> exp -> paired fp8 ew tile."""
        jt, pof = h // 2, CH * (h % 2)
        qh = q_sb[pof : pof + CH, b, jt, :]
        kh = k_sb[pof : pof + CH, b, jt, :]
        ew = ew_pool.tile([P, 2, T], FP8, tag="ew")
        for i in range(2):
            s = 2 * s2 + i
            sps = psA.tile([P, T], F32, tag="mm")
            for n in range(2):
                nc.tensor.matmul(
                    out=sps[:, 512 * n : 512 * (n + 1)],
                    lhsT=kh[:, P * s : P * (s + 1)],
                    rhs=qh[:, 512 * n : 512 * (n + 1)],
                    start=True,
                    stop=True,
                )
            nc.scalar.activation(out=ew[:, i, :], in_=sps, func=AF.Exp, bias=ebias_sb)
        return ew

    def av_pair(b, h, s2, ew, accs):
        for n in range(2):
            nc.tensor.matmul(
                out=accs[n],
                lhsT=vt2[:, b, s2, :, VT_W * h : VT_W * h + VT_USED],
                rhs=ew[:, :, 512 * n : 512 * (n + 1)],
                start=(s2 == 0),
                stop=(s2 == 3),
                perf_mode=DR,
            )

    def normalize(b, h, accs):
        """softmax denominators live in row 64 of each acc half.

        DVE supports shifted partition bases (verified on HW): reciprocal
        reads acc partition 64 and writes partition 0 directly, and the
        normalize multiply writes partitions 64-127 for odd heads."""
        jt, pof = h // 2, CH * (h % 2)
        for n in range(2):
            acc = accs[n]
            nsl = slice(512 * n, 512 * (n + 1))
            rz = rc_pool.tile([1, 512], F32, tag="rz")
            nc.vector.reciprocal(out=rz, in_=acc[CH : CH + 1, :])
            rb = rc_pool.tile([CH, 512], F32, tag="rb")
            nc.gpsimd.partition_broadcast(out_ap=rb, in_ap=rz, channels=CH)
            nc.vector.tensor_mul(
                out=a_sb[pof : pof + CH, b, jt, nsl], in0=acc[0:CH, :], in1=rb
            )

    def proj_tile(b, m, xr_t, store_engines, act_assist=False):
        pps = psA.tile([P, T], F32, tag="mm")
        o_t = outp.tile([P, T], F32, tag="o")
        for n in range(2):
            nsl = slice(512 * n, 512 * (n + 1))
            for jp in range(2):
                nc.tensor.matmul(
                    out=pps[:, nsl],
                    lhsT=wp_sb[:, 2 * jp : 2 * jp + 2, P * m : P * (m + 1)],
                    rhs=a_sb[:, b, 2 * jp : 2 * jp + 2, nsl],
                    start=(jp == 0),
                    stop=(jp == 1),
                    perf_mode=DR,
                )
            # per-half eviction + store so the tail pipelines
            if act_assist and n == 0:
                # tail only (ACT idle after its last exp): evict on ACT,
                # residual add on Pool
                tmp = outp.tile([P, 512], F32, tag="otmp")
                nc.scalar.activation(
                    out=tmp, in_=pps[:, nsl], func=AF.Identity, scale=1.0 / WSCALE
                )
                nc.gpsimd.tensor_add(out=o_t[:, nsl], in0=tmp, in1=xr_t[:, m, nsl])
            else:
                nc.vector.scalar_tensor_tensor(
                    out=o_t[:, nsl],
                    in0=pps[:, nsl],
                    scalar=1.0 / WSCALE,
                    in1=xr_t[:, m, nsl],
                    op0=OP.mult,
                    op1=OP.add,
                )
            store_engines[n].dma_start(out=ov[b, m][:, nsl], in_=o_t[:, nsl])

    # ---------------- emission schedule ----------------
    # Startup critical chain: xf(b0) -> GN stats -> xh (ACT Identity) ->
    # QK m0/m4 (ACT Identity evicts) -> first softmax exp.  DVE meanwhile
    # works through the b0 V / remaining Q,K evictions in deadline order;
    # b1's GN finisher, xh (Pool) and qkv run as stream units.
    M_ORDER = [0, 4, 1, 5, 2, 6, 3, 7]  # Q/K tile order: head h needs (h//2, 4+h//2)

    m2_a = gn_stats(0, [0, 1])
    gn_finish(0, [0, 1], m2_a, chain_eng="pool")
    xh_evict(0, 0, "scalar")
    xh_evict(0, 1, "vector")
    m2_b = gn_stats(0, [2, 3])
    gn_finish(0, [2, 3], m2_b, chain_eng="pool")
    xh_evict(0, 2, "pool")
    xh_evict(0, 3, "scalar")
    qk_tile(0, 0, "scalar")
    qk_tile(0, 4, "vector")
    for s in range(3):
        v_tile(0, s, stream=True)  # psB: keeps the psA S rotation clean

    xr0 = outp.tile([P, 4, T], F32, tag="xr", bufs=1)
    xr1 = outp.tile([P, 4, T], F32, tag="xr1", bufs=1)

    # Flat attention pipeline over all (batch, head) pairs.  AV matmuls lag
    # the QK/exp stream by AV_LAG pairs (PE is in-order: an AV waiting on its
    # exp would block the next QK matmul and starve ACT).  Interleaved units
    # are emitted BEFORE the lagged AV/normalize so their PSUM evictions sit
    # ahead of the normalize ops in the DVE queue.
    heads = [(0, h) for h in range(NH)] + [(1, h) for h in range(NH)]
    AV_LAG = 3
    NPAIR = 4 * len(heads)
    pending = {}  # pair index -> (b, h, s2, ew)
    accs_of = {}  # head index -> acc tiles

    units = {}
    units[0] = lambda: qk_tile(0, 1, stream=True)
    units[1] = lambda: qk_tile(0, 5, stream=True)
    for s in range(3, 8):  # v(0) s3..s7: evicted just ahead of their AV pair
        units[s - 1] = lambda s=s: v_tile(0, s, stream=True)
    units[7] = lambda: qk_tile(0, 2, stream=True)
    units[8] = lambda: qk_tile(0, 6, stream=True)
    units[9] = lambda: qk_tile(0, 3, stream=True)
    units[10] = lambda: qk_tile(0, 7, stream=True)
    # b1 GroupNorm: stats split into 1-tile units, finisher; xh on Pool
    gn1_state = {}
    for u, js in enumerate(([0], [1], [2], [3])):
        units[11 + u] = lambda js=js: gn1_state.setdefault(
            js[0], gn_stats(1, js)
        )
    units[15] = lambda: gn_finish(1, [0, 1, 2, 3], _merge_m2(gn1_state), stream=True)
    units[16] = lambda: tuple(xh_evict(1, j, "pool") for j in range(4))
    for j, (kind, idx) in enumerate(
        [("qk", m) for m in M_ORDER] + [("v", s) for s in range(8)]
    ):
        units[17 + j] = lambda k=kind, i=idx: (
            qk_tile(1, i, stream=True) if k == "qk" else v_tile(1, i, stream=True)
        )
    units[33] = lambda: nc.sync.dma_start(out=xr0, in_=xv[0])
    units[59] = lambda: nc.sync.dma_start(out=xr1, in_=xv[1])

    def proj_half(b, m, n, state, xr_t, store_engine):
        if n == 0:
            state[m] = outp.tile([P, T], F32, tag="o", name=f"o_{b}_{m}")
        o_t = state[m]
        pps = psB.tile([P, 512], F32, tag="u", name=f"pps_{b}_{m}_{n}")
        nsl = slice(512 * n, 512 * (n + 1))
        for jp in range(2):
            nc.tensor.matmul(
                out=pps,
                lhsT=wp_sb[:, 2 * jp : 2 * jp + 2, P * m : P * (m + 1)],
                rhs=a_sb[:, b, 2 * jp : 2 * jp + 2, nsl],
                start=(jp == 0),
                stop=(jp == 1),
                perf_mode=DR,
            )
        nc.vector.scalar_tensor_tensor(
            out=o_t[:, nsl],
            in0=pps,
            scalar=1.0 / WSCALE,
            in1=xr_t[:, m, nsl],
            op0=OP.mult,
            op1=OP.add,
        )
        store_engine.dma_start(out=ov[b, m][:, nsl], in_=o_t[:, nsl])

    proj0_state = {}
    for j in range(4):
        units[40 + 5 * j] = lambda m=j: proj_half(0, m, 0, proj0_state, xr0, nc.sync)
        units[41 + 5 * j] = lambda m=j: proj_half(0, m, 1, proj0_state, xr0, nc.sync)

    def _merge_m2(state):
        # gn_stats on [j] writes a [P,2] m2 each; build the [P,8] layout
        # _gn_phase2 expects: cols 0:4 means, 4:8 E[x^2]
        m2 = gn_pool.tile([P, 8], F32, tag="m2m", name="m2_merged")
        for j in range(4):
            nc.vector.tensor_copy(out=m2[:, j : j + 1], in_=state[j][:, 0:1])
            nc.vector.tensor_copy(out=m2[:, 4 + j : 5 + j], in_=state[j][:, 1:2])
        return m2

    def emit_av(g):
        b, h, s2, ew = pending.pop(g)
        if s2 == 0:
            accs_of[g // 4] = [
                psC.tile([VT_USED, 512], F32, tag="av", name=f"acc_{b}_{h}_{n}")
                for n in range(2)
            ]
        av_pair(b, h, s2, ew, accs_of[g // 4])
        if s2 == 3:
            normalize(b, h, accs_of.pop(g // 4))

    for g in range(NPAIR):
        hb, s2 = heads[g // 4], g % 4
        pending[g] = (*hb, s2, qk_pair(*hb, s2))
        if g in units:
            units[g]()
        if g >= AV_LAG:
            emit_av(g - AV_LAG)
    for g in range(NPAIR - AV_LAG, NPAIR):
        emit_av(g)

    for m in range(4):
        proj_tile(1, m, xr1, [nc.scalar, nc.sync], act_assist=True)

    if "dbg_xh" in ap:
        nc.sync.dma_start(out=ap["dbg_xh"].rearrange("b (j p) t -> p b j t", p=P), in_=xh)
        nc.sync.dma_start(out=ap["dbg_q"].rearrange("b (j p) t -> p b j t", p=P), in_=q_sb)
        nc.sync.dma_start(out=ap["dbg_k"].rearrange("b (j p) t -> p b j t", p=P), in_=k_sb)
        nc.sync.dma_start(
            out=ap["dbg_vt"].rearrange("b s2 i (p w) -> p b s2 i w", p=P), in_=vt2
        )
        nc.sync.dma_start(out=ap["dbg_a"].rearrange("b (j p) t -> p b j t", p=P), in_=a_sb)


def build(num_devices=NCORES, debug=False, debug_taps=False):
    from concourse import bacc

    nc = bacc.Bacc(
        "TRN2", target_bir_lowering=False, debug=debug, num_devices=num_devices
    )
    ap = {}

    def inp(name, shape, dt=F32):
        ap[name] = nc.dram_tensor(name, shape, dt, kind="ExternalInput").ap()

    inp("xr", [BL, C, T])
    inp("xbf", [BL, C, T], BF16)
    inp("wqkvT", [C, 3 * C], FP8)
    inp("wprojT", [C, C], FP8)
    inp("bqk", [P, 8])
    inp("gscale", [P, 4])
    inp("gbias", [P, 4])
    inp("gsel", [P, 8])
    inp("gexp", [8, P])
    out_ap = nc.dram_tensor("out", [BL, C, T], F32, kind="ExternalOutput").ap()
    if debug_taps:
        for nm, shape, dt in [
            ("dbg_xh", [BL, C, T], FP8),
            ("dbg_q", [BL, C, T], BF16),
            ("dbg_k", [BL, C, T], BF16),
            ("dbg_vt", [BL, 4, 2, P * NH * VT_W], FP8),
            ("dbg_a", [BL, C, T], FP8),
        ]:
            ap[nm] = nc.dram_tensor(nm, shape, dt, kind="ExternalOutput").ap()

    with tile.TileContext(nc) as tc:
        with ExitStack() as ctx:
            tc._ctx = ctx
            _kernel_body(nc, tc, ap, out_ap)
    nc.compile()
    return nc


def host_prep(x, gn_scale, gn_bias, w_qkv, b_qkv, w_proj, b_proj):
    """Shared (weight) arrays + per-batch residual/bf16 x arrays."""
    import ml_dtypes

    xr = np.ascontiguousarray(np.asarray(x, np.float32).reshape(B, C, T))
    w_qkv = np.asarray(w_qkv, np.float32)
    b_qkv = np.asarray(b_qkv, np.float32)
    w_proj = np.asarray(w_proj, np.float32)
    b_proj = np.asarray(b_proj, np.float32)
    # permute interleaved [head, (q,k,v), ch] rows -> [(q,k,v), head, ch]
    perm = np.array(
        [h * 3 * CH + w * CH + c for w in range(3) for h in range(NH) for c in range(CH)],
        dtype=np.int64,
    )
    wq_p = w_qkv[perm].copy()
    bq_p = b_qkv[perm].copy()
    wq_p[:C] *= 0.125  # attention scale (1/sqrt(sqrt(ch)))^2 folded into Q
    bq_p[:C] *= 0.125
    bv = bq_p[2 * C :]  # V bias: folded into the residual via W_p @ bv
    # residual pre-bias: out = proj(a) + (x + b_proj + W_p @ bv)
    resid_bias = b_proj + w_proj @ bv
    xresid = xr + resid_bias[None, :, None].astype(np.float32)

    shared = {
        "wqkvT": np.ascontiguousarray((wq_p * WSCALE).T).astype(
            ml_dtypes.float8_e4m3
        ),
        "wprojT": np.ascontiguousarray((w_proj * WSCALE).T).astype(
            ml_dtypes.float8_e4m3
        ),
        "bqk": np.ascontiguousarray(bq_p[: 2 * C].reshape(8, P).T),
        "gscale": np.ascontiguousarray(gn_scale.reshape(4, P).T.astype(np.float32)),
        "gbias": np.ascontiguousarray(gn_bias.reshape(4, P).T.astype(np.float32)),
        "gsel": np.ascontiguousarray(
            (np.arange(P)[:, None] // GS == np.arange(8)[None, :]).astype(np.float32)
            / GS
        ),
        "gexp": np.ascontiguousarray(
            (np.arange(8)[:, None] == np.arange(P)[None, :] // GS).astype(np.float32)
        ),
    }
    return xr, xresid, shared


_NC_CACHE = {}


def make_in_maps(inputs):
    import ml_dtypes

    xr, xresid, shared = host_prep(**inputs)
    xbf = xr.astype(ml_dtypes.bfloat16)
    return [
        {
            "xr": np.ascontiguousarray(xresid[i * BL : (i + 1) * BL]),
            "xbf": np.ascontiguousarray(xbf[i * BL : (i + 1) * BL]),
            **shared,
        }
        for i in range(NCORES)
    ]


def kernel(x, gn_scale, gn_bias, w_qkv, b_qkv, w_proj, b_proj):
    in_maps = make_in_maps(
        dict(
            x=x,
            gn_scale=gn_scale,
            gn_bias=gn_bias,
            w_qkv=w_qkv,
            b_qkv=b_qkv,
            w_proj=w_proj,
            b_proj=b_proj,
        )
    )
    if "nc" not in _NC_CACHE:
        _NC_CACHE["nc"] = build()
    nc = _NC_CACHE["nc"]
    res = run_bass_kernel_spmd(nc, in_maps, core_ids=list(range(NCORES)))
    out = np.concatenate([res.results[i]["out"] for i in range(NCORES)], axis=0)
    return np.ascontiguousarray(out.reshape(B, C, 32, 32).astype(np.float32))


# revision 8
# speedup vs baseline: 1.0014x; 1.0009x over previous
"""AttentionBlock (GroupNorm + 8-head attention + proj + residual) on 8 TRN2 NeuronCores.

ACT-bound pipeline design. Data-parallel over batch (2 per core, no
collectives). The per-core floor is the softmax exp stream on the scalar
(ACT) engine: 16 (batch,head) x T^2 = 16.8M exps = 128 x [128,1024]
activations ~ 133us; everything else is scheduled to hide under it.

  - ACT runs ONLY Exp (+ a few Identity/Copy ops during the idle startup
    window; all share one act table -> a single table load).  GroupNorm
    rstd avoids ACT Sqrt via a bit-hack rsqrt + Newton step on Pool/DVE.
  - All steady-state PSUM evictions run on DVE (tensor_scalar with
    per-partition AP scalars); Pool takes the GN small-op chains, the
    xh(b1) eviction, softmax-reciprocal broadcasts, and tail residual adds.
  - fp8e4 DoubleRow matmuls (0.5 cyc/row, 2x contraction per instr) for
    qkv, AV and proj; QK^T stays bf16.  DoubleRow weight slices need
    16B-aligned strides -> V^T pads each head to VT_W=66 columns.
  - exp outputs fp8 with a -2.5 logit bias (e4m3 range); softmax ratios are
    bias-invariant.  A ones-column in V^T makes the AV matmul accumulate
    softmax denominators for free; normalization uses partition-shifted DVE
    reciprocal/multiply (verified on HW) -> no SBUF-shuffle DMAs.
  - V bias and proj bias fold host-side: softmax weights sum to 1, so
    out = proj(sum w v) + (x + bp + Wp bv); the residual is pre-biased.
  - PSUM: 2x[128,1024] banks are reserved for the attention S-tiles; all
    interleaved work (qkv(b1), proj(b0), GN) evicts from a separate
    single-bank pool so it never gates the S rotation; AV accumulators
    rotate through 3 single-bank tiles.
  - Emission is one flat software-pipelined stream: 64 QK pairs with AV
    lagging 3 pairs, interleaved units (b1 GN/qkv, proj(b0), residual
    loads) placed by deadline, per-half proj tail on ACT+Pool/DVE.
"""

import numpy as np
from contextlib import ExitStack

import concourse.bass as bass
import concourse.tile as tile
from concourse import mybir
from concourse.bass_utils import run_bass_kernel_spmd

B, C, T = 16, 512, 1024
NH, CH = 8, 64
GS = 16  # channels per GroupNorm group
EPS = 1e-5
NCORES = 8
BL = B // NCORES  # batches per core
P = 128
F32 = mybir.dt.float32
BF16 = mybir.dt.bfloat16
FP8 = mybir.dt.float8e4
AF = mybir.ActivationFunctionType
OP = mybir.AluOpType
DR = mybir.MatmulPerfMode.DoubleRow

VT_W = 66  # per-head V^T columns: 64 ch + 1 ones col + 1 pad so the
# DoubleRow s-pair stride (NH*VT_W fp8 bytes) is 16B-aligned (HW requirement)
VT_USED = 65  # columns actually consumed by the AV matmul
WSCALE = 16.0  # fp8 weight scale (folded back out at PSUM eviction)
EXP_BIAS = -2.5  # logit shift for fp8 exp range; softmax-invariant


U32 = mybir.dt.uint32
RSQRT_MAGIC = 0x5F3759DF


def _gn_phase1(nc, tc, pools, xf, b, js, consts):
    """DVE-only GroupNorm stats for batch b, c-tiles `js`: returns the
    per-channel (mean | E[x^2]) tile m2."""
    gn_pool = pools["gn"]
    nj = len(js)
    bnraw = gn_pool.tile([P, nj, 2, 6], F32, tag="bnraw")
    mv = gn_pool.tile([P, nj, 2], F32, tag="mv")
    for ji, j in enumerate(js):
        for hf in range(2):
            nc.vector.bn_stats(
                out=bnraw[:, ji, hf, :], in_=xf[:, b, j, 512 * hf : 512 * (hf + 1)]
            )
        nc.vector.bn_aggr(out=mv[:, ji, :], in_=bnraw[:, ji, :, :])
    # m2: cols 0:nj per-channel mean (per c-tile), nj:2nj per-channel E[x^2]
    m2 = gn_pool.tile([P, 2 * nj], F32, tag="m2")
    nc.vector.tensor_copy(out=m2[:, 0:nj], in_=mv[:, :, 0])
    nc.vector.tensor_mul(out=m2[:, nj:], in0=mv[:, :, 0], in1=mv[:, :, 0])
    nc.vector.tensor_add(out=m2[:, nj:], in0=m2[:, nj:], in1=mv[:, :, 1])
    return m2


def _gn_phase2(
    nc, tc, pools, b, js, m2, at, bt, consts, chain_eng="pool", copy_eng="vector"
):
    """Group aggregation (PE) + rstd via bit-hack rsqrt + 1 Newton step.
    The serial small-op chain runs on Pool or DVE (chain_eng) so the two
    b0 half-chains execute in parallel.  ACT stays exp-only."""
    gn_pool, psA = pools["gn"], pools["psA"]
    gscale_sb, gbias_sb, gsel_sb, gexp_sb, eps_sb, magic_sb, nrA_sb, nrB_sb = consts
    nj = len(js)
    po = nc.gpsimd if chain_eng == "pool" else nc.vector

    # group-aggregate across the 16-channel groups (partition dim) on PE;
    # gsel carries the 1/16 group mean scaling
    psmm = pools.get("psu") or psA
    shape = [P, 512] if "psu" in pools else [P, T]
    tag = "u" if "psu" in pools else "mm"
    gst_ps = psmm.tile(shape, F32, tag=tag, name=f"gnst_{b}")
    nc.tensor.matmul(
        out=gst_ps[0:8, 0 : 2 * nj], lhsT=gsel_sb, rhs=m2, start=True, stop=True
    )
    gs = gn_pool.tile([8, 2 * nj], F32, tag="gs")  # cols 0:nj mu_g, nj: E2_g
    if copy_eng == "scalar":
        nc.scalar.activation(out=gs, in_=gst_ps[0:8, 0 : 2 * nj], func=AF.Copy)
    else:
        nc.vector.tensor_copy(out=gs, in_=gst_ps[0:8, 0 : 2 * nj])
    musq = gn_pool.tile([8, nj], F32, tag="musq")
    po.tensor_mul(out=musq, in0=gs[:, 0:nj], in1=gs[:, 0:nj])
    vpe = gn_pool.tile([8, nj], F32, tag="vpe")
    po.tensor_sub(out=vpe, in0=gs[:, nj:], in1=musq)
    po.tensor_scalar_add(out=vpe, in0=vpe, scalar1=eps_sb)
    # rstd = rsqrt(vpe): exponent bit-hack seed + 1 Newton-Raphson step.
    # Immediate-scalar tensor_scalar is not supported on Pool -> DVE.
    sh = gn_pool.tile([8, nj], U32, tag="sh")
    nc.vector.tensor_scalar(
        out=sh, in0=vpe.bitcast(U32), scalar1=1, scalar2=None, op0=OP.arith_shift_right
    )
    y0 = gn_pool.tile([8, nj], F32, tag="y0")
    po.tensor_tensor(
        out=y0.bitcast(U32), in0=magic_sb[:, 0:nj], in1=sh, op=OP.subtract
    )
    t1 = gn_pool.tile([8, nj], F32, tag="t1")
    po.tensor_mul(out=t1, in0=y0, in1=y0)
    po.tensor_mul(out=t1, in0=t1, in1=vpe)
    po.tensor_scalar(
        out=t1, in0=t1, scalar1=nrA_sb, scalar2=nrB_sb, op0=OP.mult, op1=OP.add
    )
    po.tensor_mul(out=gs[:, nj:], in0=y0, in1=t1)  # rstd into gs cols nj:
    # expand group stats (mean | rstd) back to per-channel on PE
    pc_ps = psmm.tile(shape, F32, tag=tag, name=f"gnpc_{b}")
    nc.tensor.matmul(
        out=pc_ps[:, 0 : 2 * nj], lhsT=gexp_sb, rhs=gs, start=True, stop=True
    )
    pc = gn_pool.tile([P, 2 * nj], F32, tag="pc")
    if copy_eng == "scalar":
        nc.scalar.activation(out=pc, in_=pc_ps[:, 0 : 2 * nj], func=AF.Copy)
    else:
        nc.vector.tensor_copy(out=pc, in_=pc_ps[:, 0 : 2 * nj])
    jsl = slice(js[0], js[0] + nj)
    po.tensor_mul(out=at[:, jsl], in0=pc[:, nj:], in1=gscale_sb[:, jsl])
    po.tensor_mul(out=bt[:, jsl], in0=pc[:, 0:nj], in1=at[:, jsl])
    po.tensor_sub(out=bt[:, jsl], in0=gbias_sb[:, jsl], in1=bt[:, jsl])


def _kernel_body(nc, tc, ap, out_ap):
    ctx = tc._ctx

    const = ctx.enter_context(tc.tile_pool(name="const", bufs=1))
    gn_pool = ctx.enter_context(tc.tile_pool(name="gn", bufs=2))
    qk_pool = ctx.enter_context(tc.tile_pool(name="qk", bufs=1))
    ew_pool = ctx.enter_context(tc.tile_pool(name="ew", bufs=6))
    rc_pool = ctx.enter_context(tc.tile_pool(name="rc", bufs=3))
    outp = ctx.enter_context(tc.tile_pool(name="outp", bufs=2))
    psA = ctx.enter_context(tc.tile_pool(name="psA", bufs=2, space="PSUM"))
    psB = ctx.enter_context(tc.tile_pool(name="psB", bufs=1, space="PSUM"))
    psC = ctx.enter_context(tc.tile_pool(name="psC", bufs=3, space="PSUM"))
    pools = {"gn": gn_pool, "psA": psA, "psC": psC}

    xv = ap["xr"].rearrange("b (m p) t -> b p m t", p=P)  # residual (pre-biased)
    ov = out_ap.rearrange("b (m p) t -> b m p t", p=P)
    xvr = ap["xbf"].rearrange("b (j p) t -> b p j t", p=P)

    # ------- loads: tiny GN consts first (ACT queue), x + weights on SP -------
    gsel_sb = const.tile([P, 8], F32)
    nc.scalar.dma_start(out=gsel_sb, in_=ap["gsel"])
    gexp_sb = const.tile([8, P], F32)
    nc.scalar.dma_start(out=gexp_sb, in_=ap["gexp"])
    gscale_sb = const.tile([P, 4], F32)
    nc.scalar.dma_start(out=gscale_sb, in_=ap["gscale"])
    gbias_sb = const.tile([P, 4], F32)
    nc.scalar.dma_start(out=gbias_sb, in_=ap["gbias"])
    bqk_sb = const.tile([P, 8], F32)
    nc.scalar.dma_start(out=bqk_sb, in_=ap["bqk"])

    xf = const.tile([P, BL, 4, T], BF16)
    nc.sync.dma_start(out=xf[:, 0, 0:2, :], in_=xvr[0][:, 0:2, :])
    nc.gpsimd.dma_start(out=xf[:, 0, 2:4, :], in_=xvr[0][:, 2:4, :])
    nc.scalar.dma_start(out=xf[:, 1], in_=xvr[1])

    wq_sb = const.tile([P, 4, 3 * C], FP8)  # w_qkv^T * 16: [cin_part, cin_tile, out]
    nc.sync.dma_start(out=wq_sb, in_=ap["wqkvT"].rearrange("(j p) o -> p j o", p=P))
    wp_sb = const.tile([P, 4, C], FP8)  # w_proj^T * 16
    nc.sync.dma_start(out=wp_sb, in_=ap["wprojT"].rearrange("(j p) o -> p j o", p=P))
    eps_sb = const.tile([8, 1], F32)
    nc.vector.memset(eps_sb, EPS)
    ebias_sb = const.tile([P, 1], F32)
    nc.vector.memset(ebias_sb, EXP_BIAS)
    magic_sb = const.tile([8, 4], U32)
    nc.vector.memset(magic_sb, RSQRT_MAGIC)
    nrA_sb = const.tile([8, 1], F32)
    nc.vector.memset(nrA_sb, -0.5)
    nrB_sb = const.tile([8, 1], F32)
    nc.vector.memset(nrB_sb, 1.5)
    consts = (gscale_sb, gbias_sb, gsel_sb, gexp_sb, eps_sb, magic_sb, nrA_sb, nrB_sb)

    # persistent data tiles
    xh = const.tile([P, BL, 4, T], FP8)  # normalized h
    q_sb = qk_pool.tile([P, BL, 4, T], BF16, tag="q")
    k_sb = qk_pool.tile([P, BL, 4, T], BF16, tag="k")
    # V^T, s-tile-pair major for DoubleRow AV: [p, b, s2, i, (h w)]
    vt2 = qk_pool.tile([P, BL, 4, 2, NH * VT_W], FP8, tag="vt")
    a_sb = qk_pool.tile([P, BL, 4, T], FP8, tag="a")
    at_t = [const.tile([P, 4], F32, name=f"at{b}") for b in range(BL)]
    bt_t = [const.tile([P, 4], F32, name=f"bt{b}") for b in range(BL)]

    # ones columns of V^T (softmax denominators ride the AV matmul)
    for b in range(BL):
        for s2 in range(4):
            for i in range(2):
                ones_view = vt2[:, b, s2, i, :].rearrange(
                    "p (h w) -> p h w", w=VT_W
                )[:, :, CH:VT_W]  # ones col + pad col (pad never read)
                nc.vector.memset(ones_view, 1.0)

    # ---------------- per-batch building blocks ----------------
    def gn_stats(b, js):
        return _gn_phase1(nc, tc, pools, xf, b, js, consts)

    def gn_finish(b, js, m2, stream=False, chain_eng="pool"):
        p = {**pools, "psu": psB} if stream else pools
        _gn_phase2(
            nc, tc, p, b, js, m2, at_t[b], bt_t[b], consts,
            chain_eng=chain_eng, copy_eng="vector" if stream else "scalar",
        )

    def xh_evict(b, j, engine="vector"):
        if engine == "scalar":
            # ACT Identity: free during startup, same act table as Exp
            nc.scalar.activation(
                out=xh[:, b, j, :],
                in_=xf[:, b, j, :],
                func=AF.Identity,
                bias=bt_t[b][:, j : j + 1],
                scale=at_t[b][:, j : j + 1],
            )
        else:
            eng = nc.gpsimd if engine == "pool" else nc.vector
            eng.tensor_scalar(
                out=xh[:, b, j, :],
                in0=xf[:, b, j, :],
                scalar1=at_t[b][:, j : j + 1],
                scalar2=bt_t[b][:, j : j + 1],
                op0=OP.mult,
                op1=OP.add,
            )

    def qk_tile(b, m, engine="vector", stream=False):
        """Q (m<4) or K (m>=4) output tile m: fp8 DoubleRow over cin pairs.

        Prologue tiles use the wide psA rotation; stream units use the
        single-bank psB pool per half so their evictions never gate the
        attention S-tile rotation."""
        dst = q_sb[:, b, m, :] if m < 4 else k_sb[:, b, m - 4, :]
        ps_full = None if stream else psA.tile([P, T], F32, tag="mm")
        for n in range(2):
            nsl = slice(512 * n, 512 * (n + 1))
            ps = (
                psB.tile([P, 512], F32, tag="u", name="qkps")
                if stream
                else ps_full[:, nsl]
            )
            for jp in range(2):
                nc.tensor.matmul(
                    out=ps,
                    lhsT=wq_sb[:, 2 * jp : 2 * jp + 2, P * m : P * (m + 1)],
                    rhs=xh[:, b, 2 * jp : 2 * jp + 2, nsl],
                    start=(jp == 0),
                    stop=(jp == 1),
                    perf_mode=DR,
                )
            # evict per half everywhere: lets the first QK matmul start on
            # the n=0 half while n=1 is still evicting
            src = ps if stream else ps_full[:, nsl]
            osl = nsl
            if engine == "scalar":
                nc.scalar.activation(
                    out=dst[:, osl],
                    in_=src,
                    func=AF.Identity,
                    bias=bqk_sb[:, m : m + 1],
                    scale=1.0 / WSCALE,
                )
            else:
                nc.vector.tensor_scalar(
                    out=dst[:, osl],
                    in0=src,
                    scalar1=1.0 / WSCALE,
                    scalar2=bqk_sb[:, m : m + 1],
                    op0=OP.mult,
                    op1=OP.add,
                )

    def v_tile(b, s, stream=False):
        """V^T s-block: [s 128, c 512] via DoubleRow, evict *1/16 to fp8."""
        if stream:
            ps = psB.tile([P, 512], F32, tag="u")
        else:
            ps_w = psA.tile([P, T], F32, tag="mm", name="vps")
            ps = ps_w[:, 0:C]
        for jp in range(2):
            nc.tensor.matmul(
                out=ps,
                lhsT=xh[:, b, 2 * jp : 2 * jp + 2, P * s : P * (s + 1)],
                rhs=wq_sb[:, 2 * jp : 2 * jp + 2, 2 * C : 3 * C],
                start=(jp == 0),
                stop=(jp == 1),
                perf_mode=DR,
            )
        dst = vt2[:, b, s // 2, s % 2, :].rearrange("p (h w) -> p h w", w=VT_W)[
            :, :, 0:CH
        ]
        nc.vector.tensor_scalar_mul(
            out=dst,
            in0=ps.rearrange("p (h c) -> p h c", c=CH),
            scalar1=1.0 / WSCALE,
        )

    def qk_pair(b, h, s2):
        """S^T for s-tiles (2*s2, 2*s2+1) -> exp -> paired fp8 ew tile."""
        jt, pof = h // 2, CH * (h % 2)
        qh = q_sb[pof : pof + CH, b, jt, :]
        kh = k_sb[pof : pof + CH, b, jt, :]
        ew = ew_pool.tile([P, 2, T], FP8, tag="ew")
        for i in range(2):
            s = 2 * s2 + i
            sps = psA.tile([P, T], F32, tag="mm")
            for n in range(2):
                nc.tensor.matmul(
                    out=sps[:, 512 * n : 512 * (n + 1)],
                    lhsT=kh[:, P * s : P * (s + 1)],
                    rhs=qh[:, 512 * n : 512 * (n + 1)],
                    start=True,
                    stop=True,
                )
            nc.scalar.activation(out=ew[:, i, :], in_=sps, func=AF.Exp, bias=ebias_sb)
        return ew

    def av_pair(b, h, s2, ew, accs):
        for n in range(2):
            nc.tensor.matmul(
                out=accs[n],
                lhsT=vt2[:, b, s2, :, VT_W * h : VT_W * h + VT_USED],
                rhs=ew[:, :, 512 * n : 512 * (n + 1)],
                start=(s2 == 0),
                stop=(s2 == 3),
                perf_mode=DR,
            )

    def normalize(b, h, accs):
        """softmax denominators live in row 64 of each acc half.

        DVE supports shifted partition bases (verified on HW): reciprocal
        reads acc partition 64 and writes partition 0 directly, and the
        normalize multiply writes partitions 64-127 for odd heads."""
        jt, pof = h // 2, CH * (h % 2)
        for n in range(2):
            acc = accs[n]
            nsl = slice(512 * n, 512 * (n + 1))
            rz = rc_pool.tile([1, 512], F32, tag="rz")
            nc.vector.reciprocal(out=rz, in_=acc[CH : CH + 1, :])
            rb = rc_pool.tile([CH, 512], F32, tag="rb")
            nc.gpsimd.partition_broadcast(out_ap=rb, in_ap=rz, channels=CH)
            nc.vector.tensor_mul(
                out=a_sb[pof : pof + CH, b, jt, nsl], in0=acc[0:CH, :], in1=rb
            )

    def proj_tile(b, m, xr_t, store_engines, act_assist=False):
        pps = psA.tile([P, T], F32, tag="mm")
        o_t = outp.tile([P, T], F32, tag="o")
        for n in range(2):
            nsl = slice(512 * n, 512 * (n + 1))
            for jp in range(2):
                nc.tensor.matmul(
                    out=pps[:, nsl],
                    lhsT=wp_sb[:, 2 * jp : 2 * jp + 2, P * m : P * (m + 1)],
                    rhs=a_sb[:, b, 2 * jp : 2 * jp + 2, nsl],
                    start=(jp == 0),
                    stop=(jp == 1),
                    perf_mode=DR,
                )
            # per-half eviction + store so the tail pipelines
            if act_assist and n == 0:
                # tail only (ACT idle after its last exp): evict on ACT,
                # residual add on Pool
                tmp = outp.tile([P, 512], F32, tag="otmp")
                nc.scalar.activation(
                    out=tmp, in_=pps[:, nsl], func=AF.Identity, scale=1.0 / WSCALE
                )
                nc.gpsimd.tensor_add(out=o_t[:, nsl], in0=tmp, in1=xr_t[:, m, nsl])
            else:
                nc.vector.scalar_tensor_tensor(
                    out=o_t[:, nsl],
                    in0=pps[:, nsl],
                    scalar=1.0 / WSCALE,
                    in1=xr_t[:, m, nsl],
                    op0=OP.mult,
                    op1=OP.add,
                )
            store_engines[n].dma_start(out=ov[b, m][:, nsl], in_=o_t[:, nsl])

    # ---------------- emission schedule ----------------
    # Startup critical chain: xf(b0) -> GN stats -> xh (ACT Identity) ->
    # QK m0/m4 (ACT Identity evicts) -> first softmax exp.  DVE meanwhile
    # works through the b0 V / remaining Q,K evictions in deadline order;
    # b1's GN finisher, xh (Pool) and qkv run as stream units.
    M_ORDER = [0, 4, 1, 5, 2, 6, 3, 7]  # Q/K tile order: head h needs (h//2, 4+h//2)

    m2_a = gn_stats(0, [0, 1])
    gn_finish(0, [0, 1], m2_a, chain_eng="pool")
    xh_evict(0, 0, "scalar")
    xh_evict(0, 1, "vector")
    m2_b = gn_stats(0, [2, 3])
    gn_finish(0, [2, 3], m2_b, chain_eng="pool")
    xh_evict(0, 2, "pool")
    xh_evict(0, 3, "scalar")
    qk_tile(0, 0, "scalar")
    qk_tile(0, 4, "vector")
    for s in range(3):
        v_tile(0, s, stream=True)  # psB: keeps the psA S rotation clean

    xr0 = outp.tile([P, 4, T], F32, tag="xr", bufs=1)
    xr1 = outp.tile([P, 4, T], F32, tag="xr1", bufs=1)

    # Flat attention pipeline over all (batch, head) pairs.  AV matmuls lag
    # the QK/exp stream by AV_LAG pairs (PE is in-order: an AV waiting on its
    # exp would block the next QK matmul and starve ACT).  Interleaved units
    # are emitted BEFORE the lagged AV/normalize so their PSUM evictions sit
    # ahead of the normalize ops in the DVE queue.
    heads = [(0, h) for h in range(NH)] + [(1, h) for h in range(NH)]
    AV_LAG = 3
    NPAIR = 4 * len(heads)
    pending = {}  # pair index -> (b, h, s2, ew)
    accs_of = {}  # head index -> acc tiles

    units = {}
    units[0] = lambda: qk_tile(0, 1, stream=True)
    units[1] = lambda: qk_tile(0, 5, stream=True)
    for s in range(3, 8):  # v(0) s3..s7: evicted just ahead of their AV pair
        units[s - 1] = lambda s=s: v_tile(0, s, stream=True)
    units[7] = lambda: qk_tile(0, 2, stream=True)
    units[8] = lambda: qk_tile(0, 6, stream=True)
    units[9] = lambda: qk_tile(0, 3, stream=True)
    units[10] = lambda: qk_tile(0, 7, stream=True)
    # b1 GroupNorm: stats split into 1-tile units, finisher; xh on Pool
    gn1_state = {}
    for u, js in enumerate(([0], [1], [2], [3])):
        units[11 + u] = lambda js=js: gn1_state.setdefault(
            js[0], gn_stats(1, js)
        )
    units[15] = lambda: gn_finish(1, [0, 1, 2, 3], _merge_m2(gn1_state), stream=True)
    units[16] = lambda: tuple(xh_evict(1, j, "pool") for j in range(4))
    for j, (kind, idx) in enumerate(
        [("qk", m) for m in M_ORDER] + [("v", s) for s in range(8)]
    ):
        units[17 + j] = lambda k=kind, i=idx: (
            qk_tile(1, i, stream=True) if k == "qk" else v_tile(1, i, stream=True)
        )
    units[33] = lambda: nc.sync.dma_start(out=xr0, in_=xv[0])
    units[59] = lambda: nc.sync.dma_start(out=xr1, in_=xv[1])

    def proj_half(b, m, n, state, xr_t, store_engine):
        if n == 0:
            state[m] = outp.tile([P, T], F32, tag="o", name=f"o_{b}_{m}")
        o_t = state[m]
        pps = psB.tile([P, 512], F32, tag="u", name=f"pps_{b}_{m}_{n}")
        nsl = slice(512 * n, 512 * (n + 1))
        for jp in range(2):
            nc.tensor.matmul(
                out=pps,
                lhsT=wp_sb[:, 2 * jp : 2 * jp + 2, P * m : P * (m + 1)],
                rhs=a_sb[:, b, 2 * jp : 2 * jp + 2, nsl],
                start=(jp == 0),
                stop=(jp == 1),
                perf_mode=DR,
            )
        nc.vector.scalar_tensor_tensor(
            out=o_t[:, nsl],
            in0=pps,
            scalar=1.0 / WSCALE,
            in1=xr_t[:, m, nsl],
            op0=OP.mult,
            op1=OP.add,
        )
        store_engine.dma_start(out=ov[b, m][:, nsl], in_=o_t[:, nsl])

    proj0_state = {}
    for j in range(4):
        units[40 + 5 * j] = lambda m=j: proj_half(0, m, 0, proj0_state, xr0, nc.sync)
        units[41 + 5 * j] = lambda m=j: proj_half(0, m, 1, proj0_state, xr0, nc.sync)

    def _merge_m2(state):
        # gn_stats on [j] writes a [P,2] m2 each; build the [P,8] layout
        # _gn_phase2 expects: cols 0:4 means, 4:8 E[x^2]
        m2 = gn_pool.tile([P, 8], F32, tag="m2m", name="m2_merged")
        for j in range(4):
            nc.vector.tensor_copy(out=m2[:, j : j + 1], in_=state[j][:, 0:1])
            nc.vector.tensor_copy(out=m2[:, 4 + j : 5 + j], in_=state[j][:, 1:2])
        return m2

    def emit_av(g):
        b, h, s2, ew = pending.pop(g)
        if s2 == 0:
            accs_of[g // 4] = [
                psC.tile([VT_USED, 512], F32, tag="av", name=f"acc_{b}_{h}_{n}")
                for n in range(2)
            ]
        av_pair(b, h, s2, ew, accs_of[g // 4])
        if s2 == 3:
            normalize(b, h, accs_of.pop(g // 4))

    for g in range(NPAIR):
        hb, s2 = heads[g // 4], g % 4
        pending[g] = (*hb, s2, qk_pair(*hb, s2))
        if g in units:
            units[g]()
        if g >= AV_LAG:
            emit_av(g - AV_LAG)
    for g in range(NPAIR - AV_LAG, NPAIR):
        emit_av(g)

    for m in range(4):
        proj_tile(1, m, xr1, [nc.scalar, nc.sync], act_assist=True)

    if "dbg_xh" in ap:
        nc.sync.dma_start(out=ap["dbg_xh"].rearrange("b (j p) t -> p b j t", p=P), in_=xh)
        nc.sync.dma_start(out=ap["dbg_q"].rearrange("b (j p) t -> p b j t", p=P), in_=q_sb)
        nc.sync.dma_start(out=ap["dbg_k"].rearrange("b (j p) t -> p b j t", p=P), in_=k_sb)
        nc.sync.dma_start(
            out=ap["dbg_vt"].rearrange("b s2 i (p w) -> p b s2 i w", p=P), in_=vt2
        )
        nc.sync.dma_start(out=ap["dbg_a"].rearrange("b (j p) t -> p b j t", p=P), in_=a_sb)


def build(num_devices=NCORES, debug=False, debug_taps=False):
    from concourse import bacc

    nc = bacc.Bacc(
        "TRN2", target_bir_lowering=False, debug=debug, num_devices=num_devices
    )
    ap = {}

    def inp(name, shape, dt=F32):
        ap[name] = nc.dram_tensor(name, shape, dt, kind="ExternalInput").ap()

    inp("xr", [BL, C, T])
    inp("xbf", [BL, C, T], BF16)
    inp("wqkvT", [C, 3 * C], FP8)
    inp("wprojT", [C, C], FP8)
    inp("bqk", [P, 8])
    inp("gscale", [P, 4])
    inp("gbias", [P, 4])
    inp("gsel", [P, 8])
    inp("gexp", [8, P])
    out_ap = nc.dram_tensor("out", [BL, C, T], F32, kind="ExternalOutput").ap()
    if debug_taps:
        for nm, shape, dt in [
            ("dbg_xh", [BL, C, T], FP8),
            ("dbg_q", [BL, C, T], BF16),
            ("dbg_k", [BL, C, T], BF16),
            ("dbg_vt", [BL, 4, 2, P * NH * VT_W], FP8),
            ("dbg_a", [BL, C, T], FP8),
        ]:
            ap[nm] = nc.dram_tensor(nm, shape, dt, kind="ExternalOutput").ap()

    with tile.TileContext(nc) as tc:
        with ExitStack() as ctx:
            tc._ctx = ctx
            _kernel_body(nc, tc, ap, out_ap)
    nc.compile()
    return nc


def host_prep(x, gn_scale, gn_bias, w_qkv, b_qkv, w_proj, b_proj):
    """Shared (weight) arrays + per-batch residual/bf16 x arrays."""
    import ml_dtypes

    xr = np.ascontiguousarray(np.asarray(x, np.float32).reshape(B, C, T))
    w_qkv = np.asarray(w_qkv, np.float32)
    b_qkv = np.asarray(b_qkv, np.float32)
    w_proj = np.asarray(w_proj, np.float32)
    b_proj = np.asarray(b_proj, np.float32)
    # permute interleaved [head, (q,k,v), ch] rows -> [(q,k,v), head, ch]
    perm = np.array(
        [h * 3 * CH + w * CH + c for w in range(3) for h in range(NH) for c in range(CH)],
        dtype=np.int64,
    )
    wq_p = w_qkv[perm].copy()
    bq_p = b_qkv[perm].copy()
    wq_p[:C] *= 0.125  # attention scale (1/sqrt(sqrt(ch)))^2 folded into Q
    bq_p[:C] *= 0.125
    bv = bq_p[2 * C :]  # V bias: folded into the residual via W_p @ bv
    # residual pre-bias: out = proj(a) + (x + b_proj + W_p @ bv)
    resid_bias = b_proj + w_proj @ bv
    xresid = xr + resid_bias[None, :, None].astype(np.float32)

    shared = {
        "wqkvT": np.ascontiguousarray((wq_p * WSCALE).T).astype(
            ml_dtypes.float8_e4m3
        ),
        "wprojT": np.ascontiguousarray((w_proj * WSCALE).T).astype(
            ml_dtypes.float8_e4m3
        ),
        "bqk": np.ascontiguousarray(bq_p[: 2 * C].reshape(8, P).T),
        "gscale": np.ascontiguousarray(gn_scale.reshape(4, P).T.astype(np.float32)),
        "gbias": np.ascontiguousarray(gn_bias.reshape(4, P).T.astype(np.float32)),
        "gsel": np.ascontiguousarray(
            (np.arange(P)[:, None] // GS == np.arange(8)[None, :]).astype(np.float32)
            / GS
        ),
        "gexp": np.ascontiguousarray(
            (np.arange(8)[:, None] == np.arange(P)[None, :] // GS).astype(np.float32)
        ),
    }
    return xr, xresid, shared


_NC_CACHE = {}


def make_in_maps(inputs):
    import ml_dtypes

    xr, xresid, shared = host_prep(**inputs)
    xbf = xr.astype(ml_dtypes.bfloat16)
    return [
        {
            "xr": np.ascontiguousarray(xresid[i * BL : (i + 1) * BL]),
            "xbf": np.ascontiguousarray(xbf[i * BL : (i + 1) * BL]),
            **shared,
        }
        for i in range(NCORES)
    ]


def kernel(x, gn_scale, gn_bias, w_qkv, b_qkv, w_proj, b_proj):
    in_maps = make_in_maps(
        dict(
            x=x,
            gn_scale=gn_scale,
            gn_bias=gn_bias,
            w_qkv=w_qkv,
            b_qkv=b_qkv,
            w_proj=w_proj,
            b_proj=b_proj,
        )
    )
    if "nc" not in _NC_CACHE:
        _NC_CACHE["nc"] = build()
    nc = _NC_CACHE["nc"]
    res = run_bass_kernel_spmd(nc, in_maps, core_ids=list(range(NCORES)))
    out = np.concatenate([res.results[i]["out"] for i in range(NCORES)], axis=0)
    return np.ascontiguousarray(out.reshape(B, C, 32, 32).astype(np.float32))


# revision 9
# speedup vs baseline: 1.0032x; 1.0018x over previous
"""AttentionBlock (GroupNorm + 8-head attention + proj + residual) on 8 TRN2 NeuronCores.

ACT-bound pipeline design. Data-parallel over batch (2 per core, no
collectives). The per-core floor is the softmax exp stream on the scalar
(ACT) engine: 16 (batch,head) x T^2 = 16.8M exps = 128 x [128,1024]
activations ~ 133us; everything else is scheduled to hide under it.

  - ACT runs ONLY Exp (+ a few Identity/Copy ops during the idle startup
    window; all share one act table -> a single table load).  GroupNorm
    rstd avoids ACT Sqrt via a bit-hack rsqrt + Newton step on Pool/DVE.
  - All steady-state PSUM evictions run on DVE (tensor_scalar with
    per-partition AP scalars); Pool takes the GN small-op chains, the
    xh(b1) eviction, softmax-reciprocal broadcasts, and tail residual adds.
  - fp8e4 DoubleRow matmuls (0.5 cyc/row, 2x contraction per instr) for
    qkv, AV and proj; QK^T stays bf16.  DoubleRow weight slices need
    16B-aligned strides -> V^T pads each head to VT_W=66 columns.
  - exp outputs fp8 with a -2.5 logit bias (e4m3 range); softmax ratios are
    bias-invariant.  A ones-column in V^T makes the AV matmul accumulate
    softmax denominators for free; normalization uses partition-shifted DVE
    reciprocal/multiply (verified on HW) -> no SBUF-shuffle DMAs.
  - V bias and proj bias fold host-side: softmax weights sum to 1, so
    out = proj(sum w v) + (x + bp + Wp bv); the residual is pre-biased.
  - PSUM: 2x[128,1024] banks are reserved for the attention S-tiles; all
    interleaved work (qkv(b1), proj(b0), GN) evicts from a separate
    single-bank pool so it never gates the S rotation; AV accumulators
    rotate through 3 single-bank tiles.
  - Emission is one flat software-pipelined stream: 64 QK pairs with AV
    lagging 3 pairs, interleaved units (b1 GN/qkv, proj(b0), residual
    loads) placed by deadline, per-half proj tail on ACT+Pool/DVE.
"""

import numpy as np
from contextlib import ExitStack

import concourse.bass as bass
import concourse.tile as tile
from concourse import mybir
from concourse.bass_utils import run_bass_kernel_spmd

B, C, T = 16, 512, 1024
NH, CH = 8, 64
GS = 16  # channels per GroupNorm group
EPS = 1e-5
NCORES = 8
BL = B // NCORES  # batches per core
P = 128
F32 = mybir.dt.float32
BF16 = mybir.dt.bfloat16
FP8 = mybir.dt.float8e4
AF = mybir.ActivationFunctionType
OP = mybir.AluOpType
DR = mybir.MatmulPerfMode.DoubleRow

VT_W = 66  # per-head V^T columns: 64 ch + 1 ones col + 1 pad so the
# DoubleRow s-pair stride (NH*VT_W fp8 bytes) is 16B-aligned (HW requirement)
VT_USED = 65  # columns actually consumed by the AV matmul
WSCALE = 16.0  # fp8 weight scale (folded back out at PSUM eviction)
EXP_BIAS = -2.5  # logit shift for fp8 exp range; softmax-invariant


U32 = mybir.dt.uint32
RSQRT_MAGIC = 0x5F3759DF


def _gn_phase1(nc, tc, pools, xf, b, js, consts):
    """DVE-only GroupNorm stats for batch b, c-tiles `js`: returns the
    per-channel (mean | E[x^2]) tile m2."""
    gn_pool = pools["gn"]
    nj = len(js)
    bnraw = gn_pool.tile([P, nj, 2, 6], F32, tag="bnraw")
    mv = gn_pool.tile([P, nj, 2], F32, tag="mv")
    for ji, j in enumerate(js):
        for hf in range(2):
            nc.vector.bn_stats(
                out=bnraw[:, ji, hf, :], in_=xf[:, b, j, 512 * hf : 512 * (hf + 1)]
            )
        nc.vector.bn_aggr(out=mv[:, ji, :], in_=bnraw[:, ji, :, :])
    # m2: cols 0:nj per-channel mean (per c-tile), nj:2nj per-channel E[x^2]
    m2 = gn_pool.tile([P, 2 * nj], F32, tag="m2")
    nc.vector.tensor_copy(out=m2[:, 0:nj], in_=mv[:, :, 0])
    nc.vector.tensor_mul(out=m2[:, nj:], in0=mv[:, :, 0], in1=mv[:, :, 0])
    nc.vector.tensor_add(out=m2[:, nj:], in0=m2[:, nj:], in1=mv[:, :, 1])
    return m2


def _gn_phase2(
    nc, tc, pools, b, js, m2, at, bt, consts, chain_eng="pool", copy_eng="vector"
):
    """Group aggregation (PE) + rstd via bit-hack rsqrt + 1 Newton step.
    The serial small-op chain runs on Pool or DVE (chain_eng) so the two
    b0 half-chains execute in parallel.  ACT stays exp-only."""
    gn_pool, psA = pools["gn"], pools["psA"]
    (
        gscale_sb, gbias_sb, gsel_sb, gexp_sb, eps_sb, magic_sb, nrA_sb, nrB_sb,
        one_sb,
    ) = consts
    nj = len(js)
    po = nc.gpsimd if chain_eng == "pool" else nc.vector

    # group-aggregate across the 16-channel groups (partition dim) on PE;
    # gsel carries the 1/16 group mean scaling
    psmm = pools.get("psu") or psA
    shape = [P, 512] if "psu" in pools else [P, T]
    tag = "u" if "psu" in pools else "mm"
    gst_ps = psmm.tile(shape, F32, tag=tag, name=f"gnst_{b}")
    nc.tensor.matmul(
        out=gst_ps[0:8, 0 : 2 * nj], lhsT=gsel_sb, rhs=m2, start=True, stop=True
    )
    gs = gn_pool.tile([8, 2 * nj], F32, tag="gs")  # cols 0:nj mu_g, nj: E2_g
    if copy_eng == "scalar":
        nc.scalar.activation(out=gs, in_=gst_ps[0:8, 0 : 2 * nj], func=AF.Copy)
    else:
        nc.vector.tensor_copy(out=gs, in_=gst_ps[0:8, 0 : 2 * nj])
    musq = gn_pool.tile([8, nj], F32, tag="musq")
    po.tensor_mul(out=musq, in0=gs[:, 0:nj], in1=gs[:, 0:nj])
    vpe = gn_pool.tile([8, nj], F32, tag="vpe")
    po.tensor_sub(out=vpe, in0=gs[:, nj:], in1=musq)
    po.tensor_scalar_add(out=vpe, in0=vpe, scalar1=eps_sb)
    # rstd = rsqrt(vpe): exponent bit-hack seed + 1 Newton-Raphson step.
    # The shift stays on DVE: Pool rejects immediate scalars and the
    # TensorScalarPtr AP-scalar form requires float32 scalars.
    sh = gn_pool.tile([8, nj], U32, tag="sh")
    nc.vector.tensor_scalar(
        out=sh, in0=vpe.bitcast(U32), scalar1=1, scalar2=None, op0=OP.arith_shift_right
    )
    y0 = gn_pool.tile([8, nj], F32, tag="y0")
    po.tensor_tensor(
        out=y0.bitcast(U32), in0=magic_sb[:, 0:nj], in1=sh, op=OP.subtract
    )
    t1 = gn_pool.tile([8, nj], F32, tag="t1")
    po.tensor_mul(out=t1, in0=y0, in1=y0)
    po.tensor_mul(out=t1, in0=t1, in1=vpe)
    po.tensor_scalar(
        out=t1, in0=t1, scalar1=nrA_sb, scalar2=nrB_sb, op0=OP.mult, op1=OP.add
    )
    po.tensor_mul(out=gs[:, nj:], in0=y0, in1=t1)  # rstd into gs cols nj:
    # expand group stats (mean | rstd) back to per-channel on PE
    pc_ps = psmm.tile(shape, F32, tag=tag, name=f"gnpc_{b}")
    nc.tensor.matmul(
        out=pc_ps[:, 0 : 2 * nj], lhsT=gexp_sb, rhs=gs, start=True, stop=True
    )
    pc = gn_pool.tile([P, 2 * nj], F32, tag="pc")
    if copy_eng == "scalar":
        nc.scalar.activation(out=pc, in_=pc_ps[:, 0 : 2 * nj], func=AF.Copy)
    else:
        nc.vector.tensor_copy(out=pc, in_=pc_ps[:, 0 : 2 * nj])
    jsl = slice(js[0], js[0] + nj)
    po.tensor_mul(out=at[:, jsl], in0=pc[:, nj:], in1=gscale_sb[:, jsl])
    po.tensor_mul(out=bt[:, jsl], in0=pc[:, 0:nj], in1=at[:, jsl])
    po.tensor_sub(out=bt[:, jsl], in0=gbias_sb[:, jsl], in1=bt[:, jsl])


def _kernel_body(nc, tc, ap, out_ap):
    ctx = tc._ctx

    const = ctx.enter_context(tc.tile_pool(name="const", bufs=1))
    gn_pool = ctx.enter_context(tc.tile_pool(name="gn", bufs=2))
    qk_pool = ctx.enter_context(tc.tile_pool(name="qk", bufs=1))
    ew_pool = ctx.enter_context(tc.tile_pool(name="ew", bufs=6))
    rc_pool = ctx.enter_context(tc.tile_pool(name="rc", bufs=3))
    outp = ctx.enter_context(tc.tile_pool(name="outp", bufs=2))
    psA = ctx.enter_context(tc.tile_pool(name="psA", bufs=2, space="PSUM"))
    psB = ctx.enter_context(tc.tile_pool(name="psB", bufs=1, space="PSUM"))
    psC = ctx.enter_context(tc.tile_pool(name="psC", bufs=3, space="PSUM"))
    pools = {"gn": gn_pool, "psA": psA, "psC": psC}

    xv = ap["xr"].rearrange("b (m p) t -> b p m t", p=P)  # residual (pre-biased)
    ov = out_ap.rearrange("b (m p) t -> b m p t", p=P)
    xvr = ap["xbf"].rearrange("b (j p) t -> b p j t", p=P)

    # ------- loads: tiny GN consts first (ACT queue), x + weights on SP -------
    gsel_sb = const.tile([P, 8], F32)
    nc.scalar.dma_start(out=gsel_sb, in_=ap["gsel"])
    gexp_sb = const.tile([8, P], F32)
    nc.scalar.dma_start(out=gexp_sb, in_=ap["gexp"])
    gscale_sb = const.tile([P, 4], F32)
    nc.scalar.dma_start(out=gscale_sb, in_=ap["gscale"])
    gbias_sb = const.tile([P, 4], F32)
    nc.scalar.dma_start(out=gbias_sb, in_=ap["gbias"])
    bqk_sb = const.tile([P, 8], F32)
    nc.scalar.dma_start(out=bqk_sb, in_=ap["bqk"])

    xf = const.tile([P, BL, 4, T], BF16)
    nc.sync.dma_start(out=xf[:, 0, 0:2, :], in_=xvr[0][:, 0:2, :])
    nc.gpsimd.dma_start(out=xf[:, 0, 2:4, :], in_=xvr[0][:, 2:4, :])
    nc.scalar.dma_start(out=xf[:, 1], in_=xvr[1])

    wq_sb = const.tile([P, 4, 3 * C], FP8)  # w_qkv^T * 16: [cin_part, cin_tile, out]
    nc.sync.dma_start(out=wq_sb, in_=ap["wqkvT"].rearrange("(j p) o -> p j o", p=P))
    wp_sb = const.tile([P, 4, C], FP8)  # w_proj^T * 16
    nc.sync.dma_start(out=wp_sb, in_=ap["wprojT"].rearrange("(j p) o -> p j o", p=P))
    eps_sb = const.tile([8, 1], F32)
    nc.vector.memset(eps_sb, EPS)
    ebias_sb = const.tile([P, 1], F32)
    nc.vector.memset(ebias_sb, EXP_BIAS)
    magic_sb = const.tile([8, 4], U32)
    nc.vector.memset(magic_sb, RSQRT_MAGIC)
    nrA_sb = const.tile([8, 1], F32)
    nc.vector.memset(nrA_sb, -0.5)
    nrB_sb = const.tile([8, 1], F32)
    nc.vector.memset(nrB_sb, 1.5)
    one_sb = const.tile([8, 1], U32)
    nc.vector.memset(one_sb, 1)
    consts = (
        gscale_sb, gbias_sb, gsel_sb, gexp_sb, eps_sb, magic_sb, nrA_sb, nrB_sb,
        one_sb,
    )

    # persistent data tiles
    xh = const.tile([P, BL, 4, T], FP8)  # normalized h
    q_sb = qk_pool.tile([P, BL, 4, T], BF16, tag="q")
    k_sb = qk_pool.tile([P, BL, 4, T], BF16, tag="k")
    # V^T, s-tile-pair major for DoubleRow AV: [p, b, s2, i, (h w)]
    vt2 = qk_pool.tile([P, BL, 4, 2, NH * VT_W], FP8, tag="vt")
    a_sb = qk_pool.tile([P, BL, 4, T], FP8, tag="a")
    at_t = [const.tile([P, 4], F32, name=f"at{b}") for b in range(BL)]
    bt_t = [const.tile([P, 4], F32, name=f"bt{b}") for b in range(BL)]

    # ones columns of V^T (softmax denominators ride the AV matmul)
    for b in range(BL):
        for s2 in range(4):
            for i in range(2):
                ones_view = vt2[:, b, s2, i, :].rearrange(
                    "p (h w) -> p h w", w=VT_W
                )[:, :, CH:VT_W]  # ones col + pad col (pad never read)
                nc.vector.memset(ones_view, 1.0)

    # ---------------- per-batch building blocks ----------------
    def gn_stats(b, js):
        return _gn_phase1(nc, tc, pools, xf, b, js, consts)

    def gn_finish(b, js, m2, stream=False, chain_eng="pool"):
        p = {**pools, "psu": psB} if stream else pools
        _gn_phase2(
            nc, tc, p, b, js, m2, at_t[b], bt_t[b], consts,
            chain_eng=chain_eng, copy_eng="vector" if stream else "scalar",
        )

    def xh_evict(b, j, engine="vector"):
        if engine == "scalar":
            # ACT Identity: free during startup, same act table as Exp
            nc.scalar.activation(
                out=xh[:, b, j, :],
                in_=xf[:, b, j, :],
                func=AF.Identity,
                bias=bt_t[b][:, j : j + 1],
                scale=at_t[b][:, j : j + 1],
            )
        else:
            eng = nc.gpsimd if engine == "pool" else nc.vector
            eng.tensor_scalar(
                out=xh[:, b, j, :],
                in0=xf[:, b, j, :],
                scalar1=at_t[b][:, j : j + 1],
                scalar2=bt_t[b][:, j : j + 1],
                op0=OP.mult,
                op1=OP.add,
            )

    def qk_tile(b, m, engine="vector", stream=False):
        """Q (m<4) or K (m>=4) output tile m: fp8 DoubleRow over cin pairs.

        Prologue tiles use the wide psA rotation; stream units use the
        single-bank psB pool per half so their evictions never gate the
        attention S-tile rotation."""
        dst = q_sb[:, b, m, :] if m < 4 else k_sb[:, b, m - 4, :]
        ps_full = None if stream else psA.tile([P, T], F32, tag="mm")
        for n in range(2):
            nsl = slice(512 * n, 512 * (n + 1))
            ps = (
                psB.tile([P, 512], F32, tag="u", name="qkps")
                if stream
                else ps_full[:, nsl]
            )
            for jp in range(2):
                nc.tensor.matmul(
                    out=ps,
                    lhsT=wq_sb[:, 2 * jp : 2 * jp + 2, P * m : P * (m + 1)],
                    rhs=xh[:, b, 2 * jp : 2 * jp + 2, nsl],
                    start=(jp == 0),
                    stop=(jp == 1),
                    perf_mode=DR,
                )
            # evict per half everywhere: lets the first QK matmul start on
            # the n=0 half while n=1 is still evicting
            src = ps if stream else ps_full[:, nsl]
            osl = nsl
            if engine == "scalar":
                nc.scalar.activation(
                    out=dst[:, osl],
                    in_=src,
                    func=AF.Identity,
                    bias=bqk_sb[:, m : m + 1],
                    scale=1.0 / WSCALE,
                )
            else:
                nc.vector.tensor_scalar(
                    out=dst[:, osl],
                    in0=src,
                    scalar1=1.0 / WSCALE,
                    scalar2=bqk_sb[:, m : m + 1],
                    op0=OP.mult,
                    op1=OP.add,
                )

    def v_tile(b, s, stream=False):
        """V^T s-block: [s 128, c 512] via DoubleRow, evict *1/16 to fp8."""
        if stream:
            ps = psB.tile([P, 512], F32, tag="u")
        else:
            ps_w = psA.tile([P, T], F32, tag="mm", name="vps")
            ps = ps_w[:, 0:C]
        for jp in range(2):
            nc.tensor.matmul(
                out=ps,
                lhsT=xh[:, b, 2 * jp : 2 * jp + 2, P * s : P * (s + 1)],
                rhs=wq_sb[:, 2 * jp : 2 * jp + 2, 2 * C : 3 * C],
                start=(jp == 0),
                stop=(jp == 1),
                perf_mode=DR,
            )
        dst = vt2[:, b, s // 2, s % 2, :].rearrange("p (h w) -> p h w", w=VT_W)[
            :, :, 0:CH
        ]
        nc.vector.tensor_scalar_mul(
            out=dst,
            in0=ps.rearrange("p (h c) -> p h c", c=CH),
            scalar1=1.0 / WSCALE,
        )

    def qk_pair(b, h, s2):
        """S^T for s-tiles (2*s2, 2*s2+1) -> exp -> paired fp8 ew tile."""
        jt, pof = h // 2, CH * (h % 2)
        qh = q_sb[pof : pof + CH, b, jt, :]
        kh = k_sb[pof : pof + CH, b, jt, :]
        ew = ew_pool.tile([P, 2, T], FP8, tag="ew")
        for i in range(2):
            s = 2 * s2 + i
            sps = psA.tile([P, T], F32, tag="mm")
            for n in range(2):
                nc.tensor.matmul(
                    out=sps[:, 512 * n : 512 * (n + 1)],
                    lhsT=kh[:, P * s : P * (s + 1)],
                    rhs=qh[:, 512 * n : 512 * (n + 1)],
                    start=True,
                    stop=True,
                )
            nc.scalar.activation(out=ew[:, i, :], in_=sps, func=AF.Exp, bias=ebias_sb)
        return ew

    def av_pair(b, h, s2, ew, accs):
        for n in range(2):
            nc.tensor.matmul(
                out=accs[n],
                lhsT=vt2[:, b, s2, :, VT_W * h : VT_W * h + VT_USED],
                rhs=ew[:, :, 512 * n : 512 * (n + 1)],
                start=(s2 == 0),
                stop=(s2 == 3),
                perf_mode=DR,
            )

    def normalize(b, h, accs):
        """softmax denominators live in row 64 of each acc half.

        DVE supports shifted partition bases (verified on HW): reciprocal
        reads acc partition 64 and writes partition 0 directly, and the
        normalize multiply writes partitions 64-127 for odd heads."""
        jt, pof = h // 2, CH * (h % 2)
        for n in range(2):
            acc = accs[n]
            nsl = slice(512 * n, 512 * (n + 1))
            rz = rc_pool.tile([1, 512], F32, tag="rz")
            nc.vector.reciprocal(out=rz, in_=acc[CH : CH + 1, :])
            rb = rc_pool.tile([CH, 512], F32, tag="rb")
            nc.gpsimd.partition_broadcast(out_ap=rb, in_ap=rz, channels=CH)
            nc.vector.tensor_mul(
                out=a_sb[pof : pof + CH, b, jt, nsl], in0=acc[0:CH, :], in1=rb
            )

    def proj_tile(b, m, xr_t, store_engines, act_assist=False):
        pps = psA.tile([P, T], F32, tag="mm")
        o_t = outp.tile([P, T], F32, tag="o")
        for n in range(2):
            nsl = slice(512 * n, 512 * (n + 1))
            for jp in range(2):
                nc.tensor.matmul(
                    out=pps[:, nsl],
                    lhsT=wp_sb[:, 2 * jp : 2 * jp + 2, P * m : P * (m + 1)],
                    rhs=a_sb[:, b, 2 * jp : 2 * jp + 2, nsl],
                    start=(jp == 0),
                    stop=(jp == 1),
                    perf_mode=DR,
                )
            # per-half eviction + store so the tail pipelines
            if act_assist and n == 0:
                # tail only (ACT idle after its last exp): evict on ACT,
                # residual add on Pool
                tmp = outp.tile([P, 512], F32, tag="otmp")
                nc.scalar.activation(
                    out=tmp, in_=pps[:, nsl], func=AF.Identity, scale=1.0 / WSCALE
                )
                nc.gpsimd.tensor_add(out=o_t[:, nsl], in0=tmp, in1=xr_t[:, m, nsl])
            else:
                nc.vector.scalar_tensor_tensor(
                    out=o_t[:, nsl],
                    in0=pps[:, nsl],
                    scalar=1.0 / WSCALE,
                    in1=xr_t[:, m, nsl],
                    op0=OP.mult,
                    op1=OP.add,
                )
            store_engines[n].dma_start(out=ov[b, m][:, nsl], in_=o_t[:, nsl])

    # ---------------- emission schedule ----------------
    # Startup critical chain: xf(b0) -> GN stats -> xh (ACT Identity) ->
    # QK m0/m4 (ACT Identity evicts) -> first softmax exp.  DVE meanwhile
    # works through the b0 V / remaining Q,K evictions in deadline order;
    # b1's GN finisher, xh (Pool) and qkv run as stream units.
    M_ORDER = [0, 4, 1, 5, 2, 6, 3, 7]  # Q/K tile order: head h needs (h//2, 4+h//2)

    m2_a = gn_stats(0, [0, 1])
    gn_finish(0, [0, 1], m2_a, chain_eng="pool")
    xh_evict(0, 0, "scalar")
    xh_evict(0, 1, "vector")
    m2_b = gn_stats(0, [2, 3])
    gn_finish(0, [2, 3], m2_b, chain_eng="pool")
    xh_evict(0, 2, "pool")
    xh_evict(0, 3, "scalar")
    qk_tile(0, 0, "scalar")
    qk_tile(0, 4, "vector")
    for s in range(2):
        v_tile(0, s, stream=True)  # psB: keeps the psA S rotation clean

    xr0 = outp.tile([P, 4, T], F32, tag="xr", bufs=1)
    xr1 = outp.tile([P, 4, T], F32, tag="xr1", bufs=1)

    # Flat attention pipeline over all (batch, head) pairs.  AV matmuls lag
    # the QK/exp stream by AV_LAG pairs (PE is in-order: an AV waiting on its
    # exp would block the next QK matmul and starve ACT).  Interleaved units
    # are emitted BEFORE the lagged AV/normalize so their PSUM evictions sit
    # ahead of the normalize ops in the DVE queue.
    heads = [(0, h) for h in range(NH)] + [(1, h) for h in range(NH)]
    AV_LAG = 4
    NPAIR = 4 * len(heads)
    pending = {}  # pair index -> (b, h, s2, ew)
    accs_of = {}  # head index -> acc tiles

    units = {}
    units[0] = lambda: v_tile(0, 2, stream=True)
    units[1] = lambda: qk_tile(0, 1, stream=True)
    units[2] = lambda: qk_tile(0, 5, stream=True)
    for s in range(3, 8):  # v(0) s3..s7: evicted just ahead of their AV pair
        units[s] = lambda s=s: v_tile(0, s, stream=True)
    units[8] = lambda: qk_tile(0, 2, stream=True)
    units[9] = lambda: qk_tile(0, 6, stream=True)
    units[10] = lambda: qk_tile(0, 3, stream=True)
    units[11] = lambda: qk_tile(0, 7, stream=True)
    # b1 GroupNorm: stats split into 1-tile units, finisher; xh on Pool
    gn1_state = {}
    for u, js in enumerate(([0], [1], [2], [3])):
        units[12 + u] = lambda js=js: gn1_state.setdefault(
            js[0], gn_stats(1, js)
        )
    units[16] = lambda: gn_finish(1, [0, 1, 2, 3], _merge_m2(gn1_state), stream=True)
    units[17] = lambda: tuple(xh_evict(1, j, "pool") for j in range(4))
    for j, (kind, idx) in enumerate(
        [("qk", m) for m in M_ORDER] + [("v", s) for s in range(8)]
    ):
        units[18 + j] = lambda k=kind, i=idx: (
            qk_tile(1, i, stream=True) if k == "qk" else v_tile(1, i, stream=True)
        )
    units[34] = lambda: nc.sync.dma_start(out=xr0, in_=xv[0])
    units[59] = lambda: nc.sync.dma_start(out=xr1, in_=xv[1])

    def proj_half(b, m, n, state, xr_t, store_engine):
        if n == 0:
            state[m] = outp.tile([P, T], F32, tag="o", name=f"o_{b}_{m}")
        o_t = state[m]
        pps = psB.tile([P, 512], F32, tag="u", name=f"pps_{b}_{m}_{n}")
        nsl = slice(512 * n, 512 * (n + 1))
        for jp in range(2):
            nc.tensor.matmul(
                out=pps,
                lhsT=wp_sb[:, 2 * jp : 2 * jp + 2, P * m : P * (m + 1)],
                rhs=a_sb[:, b, 2 * jp : 2 * jp + 2, nsl],
                start=(jp == 0),
                stop=(jp == 1),
                perf_mode=DR,
            )
        nc.vector.scalar_tensor_tensor(
            out=o_t[:, nsl],
            in0=pps,
            scalar=1.0 / WSCALE,
            in1=xr_t[:, m, nsl],
            op0=OP.mult,
            op1=OP.add,
        )
        store_engine.dma_start(out=ov[b, m][:, nsl], in_=o_t[:, nsl])

    proj0_state = {}
    for j in range(4):
        units[40 + 5 * j] = lambda m=j: proj_half(0, m, 0, proj0_state, xr0, nc.sync)
        units[41 + 5 * j] = lambda m=j: proj_half(0, m, 1, proj0_state, xr0, nc.sync)

    def _merge_m2(state):
        # gn_stats on [j] writes a [P,2] m2 each; build the [P,8] layout
        # _gn_phase2 expects: cols 0:4 means, 4:8 E[x^2]
        m2 = gn_pool.tile([P, 8], F32, tag="m2m", name="m2_merged")
        for j in range(4):
            nc.vector.tensor_copy(out=m2[:, j : j + 1], in_=state[j][:, 0:1])
            nc.vector.tensor_copy(out=m2[:, 4 + j : 5 + j], in_=state[j][:, 1:2])
        return m2

    def emit_av(g):
        b, h, s2, ew = pending.pop(g)
        if s2 == 0:
            accs_of[g // 4] = [
                psC.tile([VT_USED, 512], F32, tag="av", name=f"acc_{b}_{h}_{n}")
                for n in range(2)
            ]
        av_pair(b, h, s2, ew, accs_of[g // 4])
        if s2 == 3:
            normalize(b, h, accs_of.pop(g // 4))

    for g in range(NPAIR):
        hb, s2 = heads[g // 4], g % 4
        pending[g] = (*hb, s2, qk_pair(*hb, s2))
        if g in units:
            units[g]()
        if g >= AV_LAG:
            emit_av(g - AV_LAG)
    for g in range(NPAIR - AV_LAG, NPAIR):
        emit_av(g)

    for m in range(4):
        engs = [nc.scalar, nc.sync] if m % 2 == 0 else [nc.gpsimd, nc.gpsimd]
        proj_tile(1, m, xr1, engs, act_assist=True)

    if "dbg_xh" in ap:
        nc.sync.dma_start(out=ap["dbg_xh"].rearrange("b (j p) t -> p b j t", p=P), in_=xh)
        nc.sync.dma_start(out=ap["dbg_q"].rearrange("b (j p) t -> p b j t", p=P), in_=q_sb)
        nc.sync.dma_start(out=ap["dbg_k"].rearrange("b (j p) t -> p b j t", p=P), in_=k_sb)
        nc.sync.dma_start(
            out=ap["dbg_vt"].rearrange("b s2 i (p w) -> p b s2 i w", p=P), in_=vt2
        )
        nc.sync.dma_start(out=ap["dbg_a"].rearrange("b (j p) t -> p b j t", p=P), in_=a_sb)


def build(num_devices=NCORES, debug=False, debug_taps=False):
    from concourse import bacc

    nc = bacc.Bacc(
        "TRN2", target_bir_lowering=False, debug=debug, num_devices=num_devices
    )
    ap = {}

    def inp(name, shape, dt=F32):
        ap[name] = nc.dram_tensor(name, shape, dt, kind="ExternalInput").ap()

    inp("xr", [BL, C, T])
    inp("xbf", [BL, C, T], BF16)
    inp("wqkvT", [C, 3 * C], FP8)
    inp("wprojT", [C, C], FP8)
    inp("bqk", [P, 8])
    inp("gscale", [P, 4])
    inp("gbias", [P, 4])
    inp("gsel", [P, 8])
    inp("gexp", [8, P])
    out_ap = nc.dram_tensor("out", [BL, C, T], F32, kind="ExternalOutput").ap()
    if debug_taps:
        for nm, shape, dt in [
            ("dbg_xh", [BL, C, T], FP8),
            ("dbg_q", [BL, C, T], BF16),
            ("dbg_k", [BL, C, T], BF16),
            ("dbg_vt", [BL, 4, 2, P * NH * VT_W], FP8),
            ("dbg_a", [BL, C, T], FP8),
        ]:
            ap[nm] = nc.dram_tensor(nm, shape, dt, kind="ExternalOutput").ap()

    with tile.TileContext(nc) as tc:
        with ExitStack() as ctx:
            tc._ctx = ctx
            _kernel_body(nc, tc, ap, out_ap)
    nc.compile()
    return nc


def host_prep(x, gn_scale, gn_bias, w_qkv, b_qkv, w_proj, b_proj):
    """Shared (weight) arrays + per-batch residual/bf16 x arrays."""
    import ml_dtypes

    xr = np.ascontiguousarray(np.asarray(x, np.float32).reshape(B, C, T))
    w_qkv = np.asarray(w_qkv, np.float32)
    b_qkv = np.asarray(b_qkv, np.float32)
    w_proj = np.asarray(w_proj, np.float32)
    b_proj = np.asarray(b_proj, np.float32)
    # permute interleaved [head, (q,k,v), ch] rows -> [(q,k,v), head, ch]
    perm = np.array(
        [h * 3 * CH + w * CH + c for w in range(3) for h in range(NH) for c in range(CH)],
        dtype=np.int64,
    )
    wq_p = w_qkv[perm].copy()
    bq_p = b_qkv[perm].copy()
    wq_p[:C] *= 0.125  # attention scale (1/sqrt(sqrt(ch)))^2 folded into Q
    bq_p[:C] *= 0.125
    bv = bq_p[2 * C :]  # V bias: folded into the residual via W_p @ bv
    # residual pre-bias: out = proj(a) + (x + b_proj + W_p @ bv)
    resid_bias = b_proj + w_proj @ bv
    xresid = xr + resid_bias[None, :, None].astype(np.float32)

    shared = {
        "wqkvT": np.ascontiguousarray((wq_p * WSCALE).T).astype(
            ml_dtypes.float8_e4m3
        ),
        "wprojT": np.ascontiguousarray((w_proj * WSCALE).T).astype(
            ml_dtypes.float8_e4m3
        ),
        "bqk": np.ascontiguousarray(bq_p[: 2 * C].reshape(8, P).T),
        "gscale": np.ascontiguousarray(gn_scale.reshape(4, P).T.astype(np.float32)),
        "gbias": np.ascontiguousarray(gn_bias.reshape(4, P).T.astype(np.float32)),
        "gsel": np.ascontiguousarray(
            (np.arange(P)[:, None] // GS == np.arange(8)[None, :]).astype(np.float32)
            / GS
        ),
        "gexp": np.ascontiguousarray(
            (np.arange(8)[:, None] == np.arange(P)[None, :] // GS).astype(np.float32)
        ),
    }
    return xr, xresid, shared


_NC_CACHE = {}


def make_in_maps(inputs):
    import ml_dtypes

    xr, xresid, shared = host_prep(**inputs)
    xbf = xr.astype(ml_dtypes.bfloat16)
    return [
        {
            "xr": np.ascontiguousarray(xresid[i * BL : (i + 1) * BL]),
            "xbf": np.ascontiguousarray(xbf[i * BL : (i + 1) * BL]),
            **shared,
        }
        for i in range(NCORES)
    ]


def kernel(x, gn_scale, gn_bias, w_qkv, b_qkv, w_proj, b_proj):
    in_maps = make_in_maps(
        dict(
            x=x,
            gn_scale=gn_scale,
            gn_bias=gn_bias,
            w_qkv=w_qkv,
            b_qkv=b_qkv,
            w_proj=w_proj,
            b_proj=b_proj,
        )
    )
    if "nc" not in _NC_CACHE:
        _NC_CACHE["nc"] = build()
    nc = _NC_CACHE["nc"]
    res = run_bass_kernel_spmd(nc, in_maps, core_ids=list(range(NCORES)))
    out = np.concatenate([res.results[i]["out"] for i in range(NCORES)], axis=0)
    return np.ascontiguousarray(out.reshape(B, C, 32, 32).astype(np.float32))


# revision 10
# speedup vs baseline: 1.0036x; 1.0005x over previous
"""AttentionBlock (GroupNorm + 8-head attention + proj + residual) on 8 TRN2 NeuronCores.

ACT-bound pipeline design. Data-parallel over batch (2 per core, no
collectives). The per-core floor is the softmax exp stream on the scalar
(ACT) engine: 16 (batch,head) x T^2 = 16.8M exps = 128 x [128,1024]
activations ~ 133us; everything else is scheduled to hide under it.

  - ACT runs ONLY Exp (+ a few Identity/Copy ops during the idle startup
    window; all share one act table -> a single table load).  GroupNorm
    rstd avoids ACT Sqrt via a bit-hack rsqrt + Newton step on Pool/DVE.
  - All steady-state PSUM evictions run on DVE (tensor_scalar with
    per-partition AP scalars); Pool takes the GN small-op chains, the
    xh(b1) eviction, softmax-reciprocal broadcasts, and tail residual adds.
  - fp8e4 DoubleRow matmuls (0.5 cyc/row, 2x contraction per instr) for
    qkv, AV and proj; QK^T stays bf16.  DoubleRow weight slices need
    16B-aligned strides -> V^T pads each head to VT_W=66 columns.
  - exp outputs fp8 with a -2.5 logit bias (e4m3 range); softmax ratios are
    bias-invariant.  A ones-column in V^T makes the AV matmul accumulate
    softmax denominators for free; normalization uses partition-shifted DVE
    reciprocal/multiply (verified on HW) -> no SBUF-shuffle DMAs.
  - V bias and proj bias fold host-side: softmax weights sum to 1, so
    out = proj(sum w v) + (x + bp + Wp bv); the residual is pre-biased.
  - PSUM: 2x[128,1024] banks are reserved for the attention S-tiles; all
    interleaved work (qkv(b1), proj(b0), GN) evicts from a separate
    single-bank pool so it never gates the S rotation; AV accumulators
    rotate through 3 single-bank tiles.
  - Emission is one flat software-pipelined stream: 64 QK pairs with AV
    lagging 3 pairs, interleaved units (b1 GN/qkv, proj(b0), residual
    loads) placed by deadline, per-half proj tail on ACT+Pool/DVE.
"""

import numpy as np
from contextlib import ExitStack

import concourse.bass as bass
import concourse.tile as tile
from concourse import mybir
from concourse.bass_utils import run_bass_kernel_spmd

B, C, T = 16, 512, 1024
NH, CH = 8, 64
GS = 16  # channels per GroupNorm group
EPS = 1e-5
NCORES = 8
BL = B // NCORES  # batches per core
P = 128
F32 = mybir.dt.float32
BF16 = mybir.dt.bfloat16
FP8 = mybir.dt.float8e4
AF = mybir.ActivationFunctionType
OP = mybir.AluOpType
DR = mybir.MatmulPerfMode.DoubleRow

VT_W = 66  # per-head V^T columns: 64 ch + 1 ones col + 1 pad so the
# DoubleRow s-pair stride (NH*VT_W fp8 bytes) is 16B-aligned (HW requirement)
VT_USED = 65  # columns actually consumed by the AV matmul
WSCALE = 16.0  # fp8 weight scale (folded back out at PSUM eviction)
EXP_BIAS = -2.5  # logit shift for fp8 exp range; softmax-invariant


U32 = mybir.dt.uint32
RSQRT_MAGIC = 0x5F3759DF


def _gn_phase1(nc, tc, pools, xf, b, js, consts):
    """DVE-only GroupNorm stats for batch b, c-tiles `js`: returns the
    per-channel (mean | E[x^2]) tile m2."""
    gn_pool = pools["gn"]
    nj = len(js)
    bnraw = gn_pool.tile([P, nj, 2, 6], F32, tag="bnraw")
    mv = gn_pool.tile([P, nj, 2], F32, tag="mv")
    for ji, j in enumerate(js):
        for hf in range(2):
            nc.vector.bn_stats(
                out=bnraw[:, ji, hf, :], in_=xf[:, b, j, 512 * hf : 512 * (hf + 1)]
            )
        nc.vector.bn_aggr(out=mv[:, ji, :], in_=bnraw[:, ji, :, :])
    # m2: cols 0:nj per-channel mean (per c-tile), nj:2nj per-channel E[x^2]
    m2 = gn_pool.tile([P, 2 * nj], F32, tag="m2")
    nc.vector.tensor_copy(out=m2[:, 0:nj], in_=mv[:, :, 0])
    nc.vector.tensor_mul(out=m2[:, nj:], in0=mv[:, :, 0], in1=mv[:, :, 0])
    nc.vector.tensor_add(out=m2[:, nj:], in0=m2[:, nj:], in1=mv[:, :, 1])
    return m2


def _gn_phase2(
    nc, tc, pools, b, js, m2, at, bt, consts, chain_eng="pool", copy_eng="vector"
):
    """Group aggregation (PE) + rstd via bit-hack rsqrt + 1 Newton step.
    The serial small-op chain runs on Pool or DVE (chain_eng) so the two
    b0 half-chains execute in parallel.  ACT stays exp-only."""
    gn_pool, psA = pools["gn"], pools["psA"]
    (
        gscale_sb, gbias_sb, gsel_sb, gexp_sb, eps_sb, magic_sb, nrA_sb, nrB_sb,
        one_sb, two_sb,
    ) = consts
    nj = len(js)
    po = nc.gpsimd if chain_eng == "pool" else nc.vector

    # group-aggregate across the 16-channel groups (partition dim) on PE;
    # gsel carries the 1/16 group mean scaling
    psmm = pools.get("psu") or psA
    shape = [P, 512] if "psu" in pools else [P, T]
    tag = "u" if "psu" in pools else "mm"
    gst_ps = psmm.tile(shape, F32, tag=tag, name=f"gnst_{b}")
    nc.tensor.matmul(
        out=gst_ps[0:8, 0 : 2 * nj], lhsT=gsel_sb, rhs=m2, start=True, stop=True
    )
    gs = gn_pool.tile([8, 2 * nj], F32, tag="gs")  # cols 0:nj mu_g, nj: E2_g
    if copy_eng == "scalar":
        nc.scalar.activation(out=gs, in_=gst_ps[0:8, 0 : 2 * nj], func=AF.Copy)
    else:
        nc.vector.tensor_copy(out=gs, in_=gst_ps[0:8, 0 : 2 * nj])
    musq = gn_pool.tile([8, nj], F32, tag="musq")
    po.tensor_mul(out=musq, in0=gs[:, 0:nj], in1=gs[:, 0:nj])
    vpe = gn_pool.tile([8, nj], F32, tag="vpe")
    po.tensor_sub(out=vpe, in0=gs[:, nj:], in1=musq)
    po.tensor_scalar_add(out=vpe, in0=vpe, scalar1=eps_sb)
    # rstd = rsqrt(vpe): exponent bit-hack seed + 1 Newton-Raphson step.
    # The >>1 is a uint32 divide-by-2 on Pool (bit patterns are positive:
    # var+eps > 0), avoiding a DVE crossing mid-chain.
    sh = gn_pool.tile([8, nj], U32, tag="sh")
    po.tensor_tensor(
        out=sh, in0=vpe.bitcast(U32), in1=two_sb[:, 0:nj], op=OP.divide
    )
    y0 = gn_pool.tile([8, nj], F32, tag="y0")
    po.tensor_tensor(
        out=y0.bitcast(U32), in0=magic_sb[:, 0:nj], in1=sh, op=OP.subtract
    )
    t1 = gn_pool.tile([8, nj], F32, tag="t1")
    po.tensor_mul(out=t1, in0=y0, in1=y0)
    po.tensor_mul(out=t1, in0=t1, in1=vpe)
    po.tensor_scalar(
        out=t1, in0=t1, scalar1=nrA_sb, scalar2=nrB_sb, op0=OP.mult, op1=OP.add
    )
    po.tensor_mul(out=gs[:, nj:], in0=y0, in1=t1)  # rstd into gs cols nj:
    # expand group stats (mean | rstd) back to per-channel on PE
    pc_ps = psmm.tile(shape, F32, tag=tag, name=f"gnpc_{b}")
    nc.tensor.matmul(
        out=pc_ps[:, 0 : 2 * nj], lhsT=gexp_sb, rhs=gs, start=True, stop=True
    )
    pc = gn_pool.tile([P, 2 * nj], F32, tag="pc")
    if copy_eng == "scalar":
        nc.scalar.activation(out=pc, in_=pc_ps[:, 0 : 2 * nj], func=AF.Copy)
    else:
        nc.vector.tensor_copy(out=pc, in_=pc_ps[:, 0 : 2 * nj])
    jsl = slice(js[0], js[0] + nj)
    po.tensor_mul(out=at[:, jsl], in0=pc[:, nj:], in1=gscale_sb[:, jsl])
    po.tensor_mul(out=bt[:, jsl], in0=pc[:, 0:nj], in1=at[:, jsl])
    po.tensor_sub(out=bt[:, jsl], in0=gbias_sb[:, jsl], in1=bt[:, jsl])


def _kernel_body(nc, tc, ap, out_ap):
    ctx = tc._ctx

    const = ctx.enter_context(tc.tile_pool(name="const", bufs=1))
    gn_pool = ctx.enter_context(tc.tile_pool(name="gn", bufs=2))
    qk_pool = ctx.enter_context(tc.tile_pool(name="qk", bufs=1))
    ew_pool = ctx.enter_context(tc.tile_pool(name="ew", bufs=6))
    rc_pool = ctx.enter_context(tc.tile_pool(name="rc", bufs=3))
    outp = ctx.enter_context(tc.tile_pool(name="outp", bufs=2))
    psA = ctx.enter_context(tc.tile_pool(name="psA", bufs=2, space="PSUM"))
    psB = ctx.enter_context(tc.tile_pool(name="psB", bufs=1, space="PSUM"))
    psC = ctx.enter_context(tc.tile_pool(name="psC", bufs=3, space="PSUM"))
    pools = {"gn": gn_pool, "psA": psA, "psC": psC}

    xv = ap["xr"].rearrange("b (m p) t -> b p m t", p=P)  # residual (pre-biased)
    ov = out_ap.rearrange("b (m p) t -> b m p t", p=P)
    xvr = ap["xbf"].rearrange("b (j p) t -> b p j t", p=P)

    # ------- loads: tiny GN consts first (ACT queue), x + weights on SP -------
    gsel_sb = const.tile([P, 8], F32)
    nc.scalar.dma_start(out=gsel_sb, in_=ap["gsel"])
    gexp_sb = const.tile([8, P], F32)
    nc.scalar.dma_start(out=gexp_sb, in_=ap["gexp"])
    gscale_sb = const.tile([P, 4], F32)
    nc.scalar.dma_start(out=gscale_sb, in_=ap["gscale"])
    gbias_sb = const.tile([P, 4], F32)
    nc.scalar.dma_start(out=gbias_sb, in_=ap["gbias"])
    bqk_sb = const.tile([P, 8], F32)
    nc.scalar.dma_start(out=bqk_sb, in_=ap["bqk"])

    xf = const.tile([P, BL, 4, T], BF16)
    nc.sync.dma_start(out=xf[:, 0, 0:2, :], in_=xvr[0][:, 0:2, :])
    nc.gpsimd.dma_start(out=xf[:, 0, 2:4, :], in_=xvr[0][:, 2:4, :])
    nc.scalar.dma_start(out=xf[:, 1], in_=xvr[1])

    wq_sb = const.tile([P, 4, 3 * C], FP8)  # w_qkv^T * 16: [cin_part, cin_tile, out]
    nc.sync.dma_start(out=wq_sb, in_=ap["wqkvT"].rearrange("(j p) o -> p j o", p=P))
    wp_sb = const.tile([P, 4, C], FP8)  # w_proj^T * 16
    nc.sync.dma_start(out=wp_sb, in_=ap["wprojT"].rearrange("(j p) o -> p j o", p=P))
    eps_sb = const.tile([8, 1], F32)
    nc.vector.memset(eps_sb, EPS)
    ebias_sb = const.tile([P, 1], F32)
    nc.vector.memset(ebias_sb, EXP_BIAS)
    magic_sb = const.tile([8, 4], U32)
    nc.vector.memset(magic_sb, RSQRT_MAGIC)
    nrA_sb = const.tile([8, 1], F32)
    nc.vector.memset(nrA_sb, -0.5)
    nrB_sb = const.tile([8, 1], F32)
    nc.vector.memset(nrB_sb, 1.5)
    one_sb = const.tile([8, 1], U32)
    nc.vector.memset(one_sb, 1)
    two_sb = const.tile([8, 4], U32)
    nc.vector.memset(two_sb, 2)
    consts = (
        gscale_sb, gbias_sb, gsel_sb, gexp_sb, eps_sb, magic_sb, nrA_sb, nrB_sb,
        one_sb, two_sb,
    )

    # persistent data tiles
    xh = const.tile([P, BL, 4, T], FP8)  # normalized h
    q_sb = qk_pool.tile([P, BL, 4, T], BF16, tag="q")
    k_sb = qk_pool.tile([P, BL, 4, T], BF16, tag="k")
    # V^T, s-tile-pair major for DoubleRow AV: [p, b, s2, i, (h w)]
    vt2 = qk_pool.tile([P, BL, 4, 2, NH * VT_W], FP8, tag="vt")
    a_sb = qk_pool.tile([P, BL, 4, T], FP8, tag="a")
    at_t = [const.tile([P, 4], F32, name=f"at{b}") for b in range(BL)]
    bt_t = [const.tile([P, 4], F32, name=f"bt{b}") for b in range(BL)]

    # ones columns of V^T (softmax denominators ride the AV matmul)
    for b in range(BL):
        for s2 in range(4):
            for i in range(2):
                ones_view = vt2[:, b, s2, i, :].rearrange(
                    "p (h w) -> p h w", w=VT_W
                )[:, :, CH:VT_W]  # ones col + pad col (pad never read)
                nc.vector.memset(ones_view, 1.0)

    # ---------------- per-batch building blocks ----------------
    def gn_stats(b, js):
        return _gn_phase1(nc, tc, pools, xf, b, js, consts)

    def gn_finish(b, js, m2, stream=False, chain_eng="pool"):
        p = {**pools, "psu": psB} if stream else pools
        _gn_phase2(
            nc, tc, p, b, js, m2, at_t[b], bt_t[b], consts,
            chain_eng=chain_eng, copy_eng="vector" if stream else "scalar",
        )

    def xh_evict(b, j, engine="vector"):
        if engine == "scalar":
            # ACT Identity: free during startup, same act table as Exp
            nc.scalar.activation(
                out=xh[:, b, j, :],
                in_=xf[:, b, j, :],
                func=AF.Identity,
                bias=bt_t[b][:, j : j + 1],
                scale=at_t[b][:, j : j + 1],
            )
        else:
            eng = nc.gpsimd if engine == "pool" else nc.vector
            eng.tensor_scalar(
                out=xh[:, b, j, :],
                in0=xf[:, b, j, :],
                scalar1=at_t[b][:, j : j + 1],
                scalar2=bt_t[b][:, j : j + 1],
                op0=OP.mult,
                op1=OP.add,
            )

    def qk_tile(b, m, engine="vector", stream=False):
        """Q (m<4) or K (m>=4) output tile m: fp8 DoubleRow over cin pairs.

        Prologue tiles use the wide psA rotation; stream units use the
        single-bank psB pool per half so their evictions never gate the
        attention S-tile rotation."""
        dst = q_sb[:, b, m, :] if m < 4 else k_sb[:, b, m - 4, :]
        ps_full = None if stream else psA.tile([P, T], F32, tag="mm")
        for n in range(2):
            nsl = slice(512 * n, 512 * (n + 1))
            ps = (
                psB.tile([P, 512], F32, tag="u", name="qkps")
                if stream
                else ps_full[:, nsl]
            )
            for jp in range(2):
                nc.tensor.matmul(
                    out=ps,
                    lhsT=wq_sb[:, 2 * jp : 2 * jp + 2, P * m : P * (m + 1)],
                    rhs=xh[:, b, 2 * jp : 2 * jp + 2, nsl],
                    start=(jp == 0),
                    stop=(jp == 1),
                    perf_mode=DR,
                )
            # evict per half everywhere: lets the first QK matmul start on
            # the n=0 half while n=1 is still evicting
            src = ps if stream else ps_full[:, nsl]
            osl = nsl
            if engine == "scalar":
                nc.scalar.activation(
                    out=dst[:, osl],
                    in_=src,
                    func=AF.Identity,
                    bias=bqk_sb[:, m : m + 1],
                    scale=1.0 / WSCALE,
                )
            else:
                nc.vector.tensor_scalar(
                    out=dst[:, osl],
                    in0=src,
                    scalar1=1.0 / WSCALE,
                    scalar2=bqk_sb[:, m : m + 1],
                    op0=OP.mult,
                    op1=OP.add,
                )

    def v_tile(b, s, stream=False):
        """V^T s-block: [s 128, c 512] via DoubleRow, evict *1/16 to fp8."""
        if stream:
            ps = psB.tile([P, 512], F32, tag="u")
        else:
            ps_w = psA.tile([P, T], F32, tag="mm", name="vps")
            ps = ps_w[:, 0:C]
        for jp in range(2):
            nc.tensor.matmul(
                out=ps,
                lhsT=xh[:, b, 2 * jp : 2 * jp + 2, P * s : P * (s + 1)],
                rhs=wq_sb[:, 2 * jp : 2 * jp + 2, 2 * C : 3 * C],
                start=(jp == 0),
                stop=(jp == 1),
                perf_mode=DR,
            )
        dst = vt2[:, b, s // 2, s % 2, :].rearrange("p (h w) -> p h w", w=VT_W)[
            :, :, 0:CH
        ]
        nc.vector.tensor_scalar_mul(
            out=dst,
            in0=ps.rearrange("p (h c) -> p h c", c=CH),
            scalar1=1.0 / WSCALE,
        )

    def qk_pair(b, h, s2):
        """S^T for s-tiles (2*s2, 2*s2+1) -> exp -> paired fp8 ew tile."""
        jt, pof = h // 2, CH * (h % 2)
        qh = q_sb[pof : pof + CH, b, jt, :]
        kh = k_sb[pof : pof + CH, b, jt, :]
        ew = ew_pool.tile([P, 2, T], FP8, tag="ew")
        for i in range(2):
            s = 2 * s2 + i
            sps = psA.tile([P, T], F32, tag="mm")
            for n in range(2):
                nc.tensor.matmul(
                    out=sps[:, 512 * n : 512 * (n + 1)],
                    lhsT=kh[:, P * s : P * (s + 1)],
                    rhs=qh[:, 512 * n : 512 * (n + 1)],
                    start=True,
                    stop=True,
                )
            nc.scalar.activation(out=ew[:, i, :], in_=sps, func=AF.Exp, bias=ebias_sb)
        return ew

    def av_pair(b, h, s2, ew, accs):
        for n in range(2):
            nc.tensor.matmul(
                out=accs[n],
                lhsT=vt2[:, b, s2, :, VT_W * h : VT_W * h + VT_USED],
                rhs=ew[:, :, 512 * n : 512 * (n + 1)],
                start=(s2 == 0),
                stop=(s2 == 3),
                perf_mode=DR,
            )

    def normalize(b, h, accs):
        """softmax denominators live in row 64 of each acc half.

        DVE supports shifted partition bases (verified on HW): reciprocal
        reads acc partition 64 and writes partition 0 directly, and the
        normalize multiply writes partitions 64-127 for odd heads."""
        jt, pof = h // 2, CH * (h % 2)
        for n in range(2):
            acc = accs[n]
            nsl = slice(512 * n, 512 * (n + 1))
            rz = rc_pool.tile([1, 512], F32, tag="rz")
            nc.vector.reciprocal(out=rz, in_=acc[CH : CH + 1, :])
            rb = rc_pool.tile([CH, 512], F32, tag="rb")
            nc.gpsimd.partition_broadcast(out_ap=rb, in_ap=rz, channels=CH)
            nc.vector.tensor_mul(
                out=a_sb[pof : pof + CH, b, jt, nsl], in0=acc[0:CH, :], in1=rb
            )

    def proj_tile(b, m, xr_t, store_engines, act_assist=False):
        pps = psA.tile([P, T], F32, tag="mm")
        o_t = outp.tile([P, T], F32, tag="o")
        for n in range(2):
            nsl = slice(512 * n, 512 * (n + 1))
            for jp in range(2):
                nc.tensor.matmul(
                    out=pps[:, nsl],
                    lhsT=wp_sb[:, 2 * jp : 2 * jp + 2, P * m : P * (m + 1)],
                    rhs=a_sb[:, b, 2 * jp : 2 * jp + 2, nsl],
                    start=(jp == 0),
                    stop=(jp == 1),
                    perf_mode=DR,
                )
            # per-half eviction + store so the tail pipelines
            if act_assist and n == 0:
                # tail only (ACT idle after its last exp): evict on ACT,
                # residual add on Pool
                tmp = outp.tile([P, 512], F32, tag="otmp")
                nc.scalar.activation(
                    out=tmp, in_=pps[:, nsl], func=AF.Identity, scale=1.0 / WSCALE
                )
                nc.gpsimd.tensor_add(out=o_t[:, nsl], in0=tmp, in1=xr_t[:, m, nsl])
            else:
                nc.vector.scalar_tensor_tensor(
                    out=o_t[:, nsl],
                    in0=pps[:, nsl],
                    scalar=1.0 / WSCALE,
                    in1=xr_t[:, m, nsl],
                    op0=OP.mult,
                    op1=OP.add,
                )
            store_engines[n].dma_start(out=ov[b, m][:, nsl], in_=o_t[:, nsl])

    # ---------------- emission schedule ----------------
    # Startup critical chain: xf(b0) -> GN stats -> xh (ACT Identity) ->
    # QK m0/m4 (ACT Identity evicts) -> first softmax exp.  DVE meanwhile
    # works through the b0 V / remaining Q,K evictions in deadline order;
    # b1's GN finisher, xh (Pool) and qkv run as stream units.
    M_ORDER = [0, 4, 1, 5, 2, 6, 3, 7]  # Q/K tile order: head h needs (h//2, 4+h//2)

    m2_a = gn_stats(0, [0, 1])
    gn_finish(0, [0, 1], m2_a, chain_eng="pool")
    xh_evict(0, 0, "scalar")
    xh_evict(0, 1, "vector")
    m2_b = gn_stats(0, [2, 3])
    gn_finish(0, [2, 3], m2_b, chain_eng="pool")
    xh_evict(0, 2, "pool")
    xh_evict(0, 3, "scalar")
    qk_tile(0, 0, "scalar")
    qk_tile(0, 4, "vector")
    for s in range(2):
        v_tile(0, s, stream=True)  # psB: keeps the psA S rotation clean

    xr0 = outp.tile([P, 4, T], F32, tag="xr", bufs=1)
    xr1 = outp.tile([P, 4, T], F32, tag="xr1", bufs=1)

    # Flat attention pipeline over all (batch, head) pairs.  AV matmuls lag
    # the QK/exp stream by AV_LAG pairs (PE is in-order: an AV waiting on its
    # exp would block the next QK matmul and starve ACT).  Interleaved units
    # are emitted BEFORE the lagged AV/normalize so their PSUM evictions sit
    # ahead of the normalize ops in the DVE queue.
    heads = [(0, h) for h in range(NH)] + [(1, h) for h in range(NH)]
    AV_LAG = 4
    NPAIR = 4 * len(heads)
    pending = {}  # pair index -> (b, h, s2, ew)
    accs_of = {}  # head index -> acc tiles

    units = {}
    units[0] = lambda: v_tile(0, 2, stream=True)
    units[1] = lambda: qk_tile(0, 1, stream=True)
    units[2] = lambda: qk_tile(0, 5, stream=True)
    for s in range(3, 8):  # v(0) s3..s7: evicted just ahead of their AV pair
        units[s] = lambda s=s: v_tile(0, s, stream=True)
    units[8] = lambda: qk_tile(0, 2, stream=True)
    units[9] = lambda: qk_tile(0, 6, stream=True)
    units[10] = lambda: qk_tile(0, 3, stream=True)
    units[11] = lambda: qk_tile(0, 7, stream=True)
    # b1 GroupNorm: stats split into 1-tile units, finisher; xh on Pool
    gn1_state = {}
    for u, js in enumerate(([0], [1], [2], [3])):
        units[12 + u] = lambda js=js: gn1_state.setdefault(
            js[0], gn_stats(1, js)
        )
    units[16] = lambda: gn_finish(1, [0, 1, 2, 3], _merge_m2(gn1_state), stream=True)
    units[17] = lambda: tuple(xh_evict(1, j, "pool") for j in range(4))
    for j, (kind, idx) in enumerate(
        [("qk", m) for m in M_ORDER] + [("v", s) for s in range(8)]
    ):
        units[18 + j] = lambda k=kind, i=idx: (
            qk_tile(1, i, stream=True) if k == "qk" else v_tile(1, i, stream=True)
        )
    units[34] = lambda: nc.sync.dma_start(out=xr0, in_=xv[0])
    units[59] = lambda: nc.sync.dma_start(out=xr1, in_=xv[1])

    def proj_half(b, m, n, state, xr_t, store_engine):
        if n == 0:
            state[m] = outp.tile([P, T], F32, tag="o", name=f"o_{b}_{m}")
        o_t = state[m]
        pps = psB.tile([P, 512], F32, tag="u", name=f"pps_{b}_{m}_{n}")
        nsl = slice(512 * n, 512 * (n + 1))
        for jp in range(2):
            nc.tensor.matmul(
                out=pps,
                lhsT=wp_sb[:, 2 * jp : 2 * jp + 2, P * m : P * (m + 1)],
                rhs=a_sb[:, b, 2 * jp : 2 * jp + 2, nsl],
                start=(jp == 0),
                stop=(jp == 1),
                perf_mode=DR,
            )
        nc.vector.scalar_tensor_tensor(
            out=o_t[:, nsl],
            in0=pps,
            scalar=1.0 / WSCALE,
            in1=xr_t[:, m, nsl],
            op0=OP.mult,
            op1=OP.add,
        )
        store_engine.dma_start(out=ov[b, m][:, nsl], in_=o_t[:, nsl])

    proj0_state = {}
    for j in range(4):
        units[40 + 5 * j] = lambda m=j: proj_half(0, m, 0, proj0_state, xr0, nc.sync)
        units[41 + 5 * j] = lambda m=j: proj_half(0, m, 1, proj0_state, xr0, nc.sync)

    def _merge_m2(state):
        # gn_stats on [j] writes a [P,2] m2 each; build the [P,8] layout
        # _gn_phase2 expects: cols 0:4 means, 4:8 E[x^2]
        m2 = gn_pool.tile([P, 8], F32, tag="m2m", name="m2_merged")
        for j in range(4):
            nc.vector.tensor_copy(out=m2[:, j : j + 1], in_=state[j][:, 0:1])
            nc.vector.tensor_copy(out=m2[:, 4 + j : 5 + j], in_=state[j][:, 1:2])
        return m2

    def emit_av(g):
        b, h, s2, ew = pending.pop(g)
        if s2 == 0:
            accs_of[g // 4] = [
                psC.tile([VT_USED, 512], F32, tag="av", name=f"acc_{b}_{h}_{n}")
                for n in range(2)
            ]
        av_pair(b, h, s2, ew, accs_of[g // 4])
        if s2 == 3:
            normalize(b, h, accs_of.pop(g // 4))

    for g in range(NPAIR):
        hb, s2 = heads[g // 4], g % 4
        pending[g] = (*hb, s2, qk_pair(*hb, s2))
        if g in units:
            units[g]()
        if g >= AV_LAG:
            emit_av(g - AV_LAG)
    for g in range(NPAIR - AV_LAG, NPAIR):
        emit_av(g)

    for m in range(4):
        engs = [nc.scalar, nc.sync] if m % 2 == 0 else [nc.gpsimd, nc.gpsimd]
        proj_tile(1, m, xr1, engs, act_assist=True)

    if "dbg_xh" in ap:
        nc.sync.dma_start(out=ap["dbg_xh"].rearrange("b (j p) t -> p b j t", p=P), in_=xh)
        nc.sync.dma_start(out=ap["dbg_q"].rearrange("b (j p) t -> p b j t", p=P), in_=q_sb)
        nc.sync.dma_start(out=ap["dbg_k"].rearrange("b (j p) t -> p b j t", p=P), in_=k_sb)
        nc.sync.dma_start(
            out=ap["dbg_vt"].rearrange("b s2 i (p w) -> p b s2 i w", p=P), in_=vt2
        )
        nc.sync.dma_start(out=ap["dbg_a"].rearrange("b (j p) t -> p b j t", p=P), in_=a_sb)


def build(num_devices=NCORES, debug=False, debug_taps=False):
    from concourse import bacc

    nc = bacc.Bacc(
        "TRN2", target_bir_lowering=False, debug=debug, num_devices=num_devices
    )
    ap = {}

    def inp(name, shape, dt=F32):
        ap[name] = nc.dram_tensor(name, shape, dt, kind="ExternalInput").ap()

    inp("xr", [BL, C, T])
    inp("xbf", [BL, C, T], BF16)
    inp("wqkvT", [C, 3 * C], FP8)
    inp("wprojT", [C, C], FP8)
    inp("bqk", [P, 8])
    inp("gscale", [P, 4])
    inp("gbias", [P, 4])
    inp("gsel", [P, 8])
    inp("gexp", [8, P])
    out_ap = nc.dram_tensor("out", [BL, C, T], F32, kind="ExternalOutput").ap()
    if debug_taps:
        for nm, shape, dt in [
            ("dbg_xh", [BL, C, T], FP8),
            ("dbg_q", [BL, C, T], BF16),
            ("dbg_k", [BL, C, T], BF16),
            ("dbg_vt", [BL, 4, 2, P * NH * VT_W], FP8),
            ("dbg_a", [BL, C, T], FP8),
        ]:
            ap[nm] = nc.dram_tensor(nm, shape, dt, kind="ExternalOutput").ap()

    with tile.TileContext(nc) as tc:
        with ExitStack() as ctx:
            tc._ctx = ctx
            _kernel_body(nc, tc, ap, out_ap)
    nc.compile()
    return nc


def host_prep(x, gn_scale, gn_bias, w_qkv, b_qkv, w_proj, b_proj):
    """Shared (weight) arrays + per-batch residual/bf16 x arrays."""
    import ml_dtypes

    xr = np.ascontiguousarray(np.asarray(x, np.float32).reshape(B, C, T))
    w_qkv = np.asarray(w_qkv, np.float32)
    b_qkv = np.asarray(b_qkv, np.float32)
    w_proj = np.asarray(w_proj, np.float32)
    b_proj = np.asarray(b_proj, np.float32)
    # permute interleaved [head, (q,k,v), ch] rows -> [(q,k,v), head, ch]
    perm = np.array(
        [h * 3 * CH + w * CH + c for w in range(3) for h in range(NH) for c in range(CH)],
        dtype=np.int64,
    )
    wq_p = w_qkv[perm].copy()
    bq_p = b_qkv[perm].copy()
    wq_p[:C] *= 0.125  # attention scale (1/sqrt(sqrt(ch)))^2 folded into Q
    bq_p[:C] *= 0.125
    bv = bq_p[2 * C :]  # V bias: folded into the residual via W_p @ bv
    # residual pre-bias: out = proj(a) + (x + b_proj + W_p @ bv)
    resid_bias = b_proj + w_proj @ bv
    xresid = xr + resid_bias[None, :, None].astype(np.float32)

    shared = {
        "wqkvT": np.ascontiguousarray((wq_p * WSCALE).T).astype(
            ml_dtypes.float8_e4m3
        ),
        "wprojT": np.ascontiguousarray((w_proj * WSCALE).T).astype(
            ml_dtypes.float8_e4m3
        ),
        "bqk": np.ascontiguousarray(bq_p[: 2 * C].reshape(8, P).T),
        "gscale": np.ascontiguousarray(gn_scale.reshape(4, P).T.astype(np.float32)),
        "gbias": np.ascontiguousarray(gn_bias.reshape(4, P).T.astype(np.float32)),
        "gsel": np.ascontiguousarray(
            (np.arange(P)[:, None] // GS == np.arange(8)[None, :]).astype(np.float32)
            / GS
        ),
        "gexp": np.ascontiguousarray(
            (np.arange(8)[:, None] == np.arange(P)[None, :] // GS).astype(np.float32)
        ),
    }
    return xr, xresid, shared


_NC_CACHE = {}


def make_in_maps(inputs):
    import ml_dtypes

    xr, xresid, shared = host_prep(**inputs)
    xbf = xr.astype(ml_dtypes.bfloat16)
    return [
        {
            "xr": np.ascontiguousarray(xresid[i * BL : (i + 1) * BL]),
            "xbf": np.ascontiguousarray(xbf[i * BL : (i + 1) * BL]),
            **shared,
        }
        for i in range(NCORES)
    ]


def kernel(x, gn_scale, gn_bias, w_qkv, b_qkv, w_proj, b_proj):
    in_maps = make_in_maps(
        dict(
            x=x,
            gn_scale=gn_scale,
            gn_bias=gn_bias,
            w_qkv=w_qkv,
            b_qkv=b_qkv,
            w_proj=w_proj,
            b_proj=b_proj,
        )
    )
    if "nc" not in _NC_CACHE:
        _NC_CACHE["nc"] = build()
    nc = _NC_CACHE["nc"]
    res = run_bass_kernel_spmd(nc, in_maps, core_ids=list(range(NCORES)))
    out = np.concatenate([res.results[i]["out"] for i in range(NCORES)], axis=0)
    return np.ascontiguousarray(out.reshape(B, C, 32, 32).astype(np.float32))
